# revision 18
# baseline (speedup 1.0000x reference)
"""Trainium2 Bass kernel for nn_Attention_62414464746139.

Full causal attention layer: QKV projection + RoPE + causal softmax
attention + output projection.  B=4, T=2048, C=2048, H=16, D=128, f32.

Sharding over 8 NeuronCores: core c handles batch b = c//2 and head
group g = c%2 (8 of the 16 heads).  Each core computes its heads' QKV
projection, attention, and a *partial* output projection (its heads'
rows of Wout); the host sums the two head-group partials per batch.

Fully SBUF-resident dataflow (no DRAM scratch):
  - all weights/activations host-prepped into partition-major layouts so
    every DMA moves >=1KB contiguous lines per partition
  - v computed first (xT stationary, Wv moving), stored [t, f] in SBUF
  - per head h: q,k projected into [d, t] SBUF tiles; RoPE applied on
    the full [128, T] tile (rotate-half = SBUF->SBUF half-swap DMA with
    the sign folded into the +-sin table; 3 full-width DVE ops)
  - attention per head: ST[t2, t1] = kT.T @ qT in 3-block PSUM groups
    [128, 1536]; ONE exp activation per group (with a 2^-12 bias folded
    in so fp16 row sums cannot overflow; the scale cancels in the
    normalize); softmax denominator via DVE accumulation of e into esum
    [128, 512] + ONE ones-matmul per chunk (partition reduce);
    yT[d, t1] = v.T @ e.  Scores/y matmuls and esum adds are narrowed
    to the exact causal column range; the masked-out e region holds
    exp of stale PSUM that no consumer reads (no memsets).
  - emission interleaves head h-1's attention into head h's projection
    so ACT/DVE/GPSIMD work hides under projection matmuls
  - out projection reads yT from SBUF; outT written as fp16 (host sums
    the two head-group partials in f32).

Matmul operands are float16 (full PE rate); PSUM accumulation fp32.
Softmax is computed without max subtraction: scores*scale are O(10), so
exp stays within fp16 range after the 2^-12 bias.
"""

import math

import numpy as np

import bass_rust

import concourse.bacc as bacc
import concourse.bass as bass
import concourse.mybir as mybir
import concourse.tile as tile
from concourse.bass_utils import run_bass_kernel_spmd

B, T, C = 4, 2048, 2048
H, D = 16, 128
HPC = 8            # heads per core
F = HPC * D        # 1024: per-core feature width
NCORES = 8
THETA = 10000.0
SCALE = 1.0 / math.sqrt(D)
NEG = -1.0e30
LOG_ALPHA = -12.0 * math.log(2.0)   # exp bias: e' = 2^-12 * exp(s*scale)

F32 = mybir.dt.float32
F16 = mybir.dt.float16

TCH = T // 512     # 4  t-chunks of 512
CCH = C // 128     # 16 c-chunks of 128
TT = T // 128      # 16 t-tiles of 128


def _rope_tables():
    inv_freq = 1.0 / (THETA ** (np.arange(0, D, 2, dtype=np.float32) / D))
    pos = np.arange(T, dtype=np.float32)
    freqs = np.outer(pos, inv_freq).astype(np.float32)          # [T, D/2]
    emb = np.concatenate([freqs, freqs], axis=-1)               # [T, D]
    cosT = np.cos(emb).T.astype(np.float16).copy()              # [D, T]
    sinT = np.sin(emb).T.astype(np.float16).copy()
    # rotate-half is a plain half-swap if the sign lives in the table:
    # o = raw*cos + swap(raw)*sinS with sinS[:64] = -sin[:64].
    sinS = sinT.copy()
    sinS[: D // 2] = -sinS[: D // 2]
    return cosT, sinS


def _build_program(phases=("a", "b", "c"), repeat=1):
    nc = bacc.Bacc("TRN2", target_bir_lowering=False, debug=False)

    xTr = nc.dram_tensor("xT", [128, CCH, T], F16, kind="ExternalInput")
    wqr = nc.dram_tensor("wq", [128, HPC, CCH, 128], F16,
                         kind="ExternalInput")
    wkr = nc.dram_tensor("wk", [128, HPC, CCH, 128], F16,
                         kind="ExternalInput")
    wvr = nc.dram_tensor("wv", [128, CCH, F], F16, kind="ExternalInput")
    wor = nc.dram_tensor("wout", [128, HPC, C], F16, kind="ExternalInput")
    outT = nc.dram_tensor("outT", [C, T], F16, kind="ExternalOutput")

    cosT_np, sinS_np = _rope_tables()
    r, c = np.arange(128)[:, None], np.arange(128)[None, :]
    trimask_np = np.where(r <= c, 0.0, NEG).astype(np.float32)

    cosT_d = nc.inline_tensor(cosT_np, name="cosT")
    sinS_d = nc.inline_tensor(sinS_np, name="sinS")
    trimask_d = nc.inline_tensor(trimask_np, name="trimask")
    bias_d = nc.inline_tensor(
        np.full((128, 1), LOG_ALPHA, dtype=np.float32), name="ebias")
    ones_d = nc.inline_tensor(np.ones((128, 128), dtype=np.float16),
                              name="onesm")

    with tile.TileContext(nc) as tc, \
         nc.allow_low_precision(reason="fp16 softmax row sums, 2^-12 biased"):
        with tc.tile_pool(name="consts", bufs=1) as consts:
            cosT_sb = consts.tile([D, T], F16)
            sinS_sb = consts.tile([D, T], F16)
            trimask = consts.tile([128, 128], F32)
            ebias = consts.tile([128, 1], F32)
            onesv = consts.tile([128, 128], F16)
            nc.scalar.dma_start(out=cosT_sb[:], in_=cosT_d[:])
            nc.scalar.dma_start(out=sinS_sb[:], in_=sinS_d[:])
            nc.scalar.dma_start(out=trimask[:], in_=trimask_d[:])
            nc.scalar.dma_start(out=ebias[:], in_=bias_d[:])
            nc.scalar.dma_start(out=onesv[:], in_=ones_d[:])

            def run_phases():
                _emit_iteration(nc, tc, phases,
                                xTr, wqr, wkr, wvr, wor, outT,
                                cosT_sb, sinS_sb, trimask, ebias, onesv)

            if repeat == 1:
                run_phases()
            else:
                with tc.For_i(0, repeat, 1):
                    run_phases()
    if not globals().get('_DISABLE_ELIDE'):
        _elide_redundant_ldweights(nc)
    nc.finalize()
    return nc


def _emit_iteration(nc, tc, phases, xTr, wqr, wkr, wvr, wor, outT,
                    cosT_sb, sinS_sb, trimask, ebias, onesv):
    with tc.tile_pool(name="qk", bufs=2) as qk, \
         tc.tile_pool(name="vpool", bufs=1) as vpool, \
         tc.tile_pool(name="ypool", bufs=1) as ypool, \
         tc.tile_pool(name="epool", bufs=2) as epool, \
         tc.tile_pool(name="drain", bufs=2) as drain:

        v_sb = vpool.tile([128, TT, F], F16)      # [t, tt, f]
        yT_sb = ypool.tile([128, HPC, T], F16)    # [d, head, t]
        head_tiles = {}
        shared = {"last_drain": None}

        # ---------------- attention quanta for one head ---------------
        # Per t1-chunk j (512 cols), t2 blocks 0..4(j+1)-1 are processed
        # in groups of <=3 (one 3-bank PSUM score tile + one exp each).
        # Diagonal blocks (i >= 4j) narrow the scores/y matmuls and the
        # esum adds to cols >= c0; the masked e region is stale garbage
        # that no consumer ever reads (no memsets needed).
        # Softmax denominator: e blocks are accumulated across k-blocks
        # on the DVE into esum [t2=128, t1=512]; ONE ones-matmul per
        # chunk partition-reduces esum (replaces the per-block PE sums
        # matmuls of the previous version).
        def b_quanta(h, bst, byp, bsm):
            qTh, kTh = head_tiles[h]

            # one factory call per chunk so every closure gets its own
            # scope (a plain `for j` loop would rebind the inner defs and
            # leave all quanta pointing at the last chunk's state)
            def make_chunk(j):
                quanta = []
                nblk = 4 * (j + 1)
                groups = [list(range(s, min(s + 2, nblk)))
                          for s in range(0, nblk, 2)]
                state = {}

                def c0_of(i):
                    return max(0, 128 * i - 512 * j)

                def start_chunk():
                    state["yps"] = byp.tile([128, 512], F32, tag="yps",
                                            name="yps")
                    state["esum"] = epool.tile([128, 512], F16, tag="esum",
                                               name="esum")

                def group(blocks):
                    stile = bst.tile([128, 1024], F32, tag="st")
                    e = epool.tile([128, 1024], F16, tag="e")
                    for b, i in enumerate(blocks):
                        c0 = c0_of(i)
                        nc.tensor.matmul(
                            stile[:, b * 512 + c0:(b + 1) * 512],
                            kTh[:, i * 128:(i + 1) * 128],
                            qTh[:, j * 512 + c0:(j + 1) * 512],
                            start=True, stop=True)
                        if i >= 4 * j:
                            nc.vector.tensor_add(
                                stile[:, b * 512 + c0:b * 512 + c0 + 128],
                                stile[:, b * 512 + c0:b * 512 + c0 + 128],
                                trimask[:])
                    # exp over the full group width: cols [b*512, b*512+c0)
                    # of diagonal blocks are exp of stale PSUM, but every
                    # downstream read (esum add, y matmul) narrows past c0.
                    w = 512 * len(blocks)
                    nc.scalar.activation(
                        e[:, 0:w], stile[:, 0:w],
                        mybir.ActivationFunctionType.Exp,
                        bias=ebias[:], scale=SCALE)
                    esum = state["esum"]
                    for b, i in enumerate(blocks):
                        c0 = c0_of(i)
                        if i == 0:
                            nc.vector.tensor_copy(
                                esum[:], e[:, 0:512])
                        else:
                            nc.vector.tensor_add(
                                esum[:, c0:], esum[:, c0:],
                                e[:, b * 512 + c0:(b + 1) * 512])
                    state["e"] = e
                    state["eblocks"] = blocks

                def ymm():
                    e = state["e"]
                    for b, i in enumerate(state["eblocks"]):
                        c0 = c0_of(i)
                        mm = nc.tensor.matmul(
                            state["yps"][:, c0:],
                            v_sb[:, i, h * 128:(h + 1) * 128],
                            e[:, b * 512 + c0:(b + 1) * 512],
                            start=(i == 0), stop=(i == nblk - 1),
                            skip_group_check=True)
                        if i == 0 and shared["last_drain"]:
                            # WAR: byp is single-buffered and tile does
                            # not track the drain's PSUM read
                            mm.ins.add_dependency(
                                shared["last_drain"],
                                bass_rust.DependencyInfo.SYNC_ONLY)

                def drain_chunk():
                    sums = bsm.tile([128, 512], F32, tag="sums",
                                    name="sums")
                    nc.tensor.matmul(sums[:], onesv[:], state["esum"][:],
                                     start=True, stop=True)
                    # sums >= the diagonal term 2^-12*exp(|q|^2*scale),
                    # which is far above fp16/f32 underflow: no clamp.
                    recip = drain.tile([128, 512], F32, tag="recip",
                                       bufs=1)
                    nc.vector.reciprocal(recip[:], sums[:])
                    mul = nc.vector.tensor_mul(
                        yT_sb[:, h, j * 512:(j + 1) * 512],
                        state["yps"][:], recip[:])
                    shared["last_drain"] = mul.ins.name

                for g in range(len(groups)):
                    def q_g(g=g):
                        if g == 0:
                            start_chunk()
                        else:
                            ymm()
                        group(groups[g])
                    quanta.append(q_g)

                def q_last():
                    ymm()
                    drain_chunk()
                quanta.append(q_last)
                return quanta

            return [make_chunk(j) for j in range(TCH)]

        # ------------------------- phase A2: v -------------------------
        # cc-outer / f-half-inner so both matmuls of a cc step share the
        # xT-block stationary (second load elided).
        def emit_a2(xT_sb, wvres, vps):
            wvt = wvres.tile([128, CCH, F], F16)
            for jh in range(2):
                nc.gpsimd.dma_start(out=wvt[:, :, jh * 512:(jh + 1) * 512],
                                    in_=wvr[:, :, jh * 512:(jh + 1) * 512])
            for tt in range(TT):
                ps0 = vps.tile([128, 512], F32, tag="vps0", name="ps0",
                               bufs=2)
                ps1 = vps.tile([128, 512], F32, tag="vps1", name="ps1",
                               bufs=2)
                for cc in range(CCH):
                    xblk = xT_sb[:, cc, tt * 128:(tt + 1) * 128]
                    nc.tensor.matmul(
                        ps0[:], xblk, wvt[:, cc, 0:512],
                        start=(cc == 0), stop=(cc == CCH - 1))
                    nc.tensor.matmul(
                        ps1[:], xblk, wvt[:, cc, 512:1024],
                        start=(cc == 0), stop=(cc == CCH - 1))
                nc.scalar.copy(v_sb[:, tt, 0:512], ps0[:])
                nc.scalar.copy(v_sb[:, tt, 512:1024], ps1[:])

        # ----------------- phase A1 quanta for one head ----------------
        whold = {}

        def load_w(wstream, dram, h, nm):
            wt = wstream.tile([128, CCH, 128], F16, tag="w", name=nm)
            nc.sync.dma_start(out=wt[:], in_=dram[:, h])
            return wt

        def a1_quanta(wstream, xT_sb, aps, aev, h):
            wqh, wkh = whold.pop(h)
            qTh = qk.tile([128, T], F16, tag="qT")
            kTh = qk.tile([128, T], F16, tag="kT")
            head_tiles[h] = (qTh, kTh)

            def make_tensor(idx, wt, dst, dram):
                state = {}
                quanta = []

                def start_tensor():
                    state["rawT"] = aev.tile([128, T], F16, tag="raw",
                                             name="rawT", bufs=1)

                # chunk pairs share the wq/wk-block stationary per cc
                # step (second load elided)
                for pair in range(TCH // 2):
                    def unit(pair=pair):
                        if pair == 0:
                            start_tensor()
                        ts0 = slice(1024 * pair, 1024 * pair + 512)
                        ts1 = slice(1024 * pair + 512, 1024 * pair + 1024)
                        ps0 = aps.tile([128, 512], F32, tag="aps0",
                                       name="ps0", bufs=2)
                        ps1 = aps.tile([128, 512], F32, tag="aps1",
                                       name="ps1", bufs=2)
                        for cc in range(CCH):
                            wblk = wt[:, cc, :]
                            nc.tensor.matmul(
                                ps0[:], wblk, xT_sb[:, cc, ts0],
                                start=(cc == 0), stop=(cc == CCH - 1))
                            nc.tensor.matmul(
                                ps1[:], wblk, xT_sb[:, cc, ts1],
                                start=(cc == 0), stop=(cc == CCH - 1))
                        nc.scalar.copy(state["rawT"][:, ts0], ps0[:])
                        nc.scalar.copy(state["rawT"][:, ts1], ps1[:])
                    quanta.append(unit)

                def finish():
                    # prefetch this tensor's weights for head h+1 (its
                    # predecessor buffer is free now)
                    if h + 1 < HPC:
                        whold.setdefault(h + 1, [None, None])[idx] = \
                            load_w(wstream, dram, h + 1, f"w{idx}_{h + 1}")
                    rawT = state["rawT"]
                    rotT = aev.tile([128, T], F16, tag="rot", bufs=1)
                    nc.gpsimd.dma_start(out=rotT[0:64, :],
                                        in_=rawT[64:128, :])
                    nc.gpsimd.dma_start(out=rotT[64:128, :],
                                        in_=rawT[0:64, :])
                    nc.vector.tensor_mul(dst[:], rawT[:], cosT_sb[:])
                    nc.vector.tensor_mul(rotT[:], rotT[:], sinS_sb[:])
                    nc.vector.tensor_add(dst[:], dst[:], rotT[:])
                quanta.append(finish)
                return quanta

            return (make_tensor(0, wqh, qTh, wqr)
                    + make_tensor(1, wkh, kTh, wkr))

        def merge_emit(a_list, b_list):
            """Emit two quantum streams proportionally interleaved."""
            na, nb = len(a_list), len(b_list)
            ia = ib = 0
            while ia < na or ib < nb:
                if ib * max(na, 1) <= ia * max(nb, 1) and ib < nb or ia >= na:
                    b_list[ib]()
                    ib += 1
                else:
                    a_list[ia]()
                    ia += 1

        # ------------------------- main schedule -----------------------
        with tc.tile_pool(name="bst", bufs=1, space="PSUM") as bst, \
             tc.tile_pool(name="byp", bufs=1, space="PSUM") as byp, \
             tc.tile_pool(name="bsm", bufs=1, space="PSUM") as bsm:
            with tc.tile_pool(name="xres", bufs=1) as xres:
                xT_sb = xres.tile([128, CCH, T], F16)
                for q in range(TCH):
                    eng = nc.sync if q % 2 == 0 else nc.scalar
                    eng.dma_start(
                        out=xT_sb[:, :, q * 512:(q + 1) * 512],
                        in_=xTr[:, :, q * 512:(q + 1) * 512])

                if "a" in phases:
                  with tc.tile_pool(name="wstream", bufs=2) as wstream:
                    whold[0] = [load_w(wstream, wqr, 0, "wq0"),
                                load_w(wstream, wkr, 0, "wk0")]
                    with tc.tile_pool(name="wvres", bufs=1) as wvres, \
                         tc.tile_pool(name="vps", bufs=2,
                                      space="PSUM") as vps:
                        emit_a2(xT_sb, wvres, vps)

                    with tc.tile_pool(name="aps", bufs=2,
                                      space="PSUM") as aps, \
                         tc.tile_pool(name="aev", bufs=2) as aev:
                        for h in range(HPC):
                            a_list = a1_quanta(wstream, xT_sb, aps, aev, h)
                            b_list = (sum(b_quanta(h - 1, bst, byp, bsm), [])
                                      if ("b" in phases and h > 0) else [])
                            merge_emit(a_list, b_list)

            # tail: last head's attention + output projection
            with tc.tile_pool(name="cres", bufs=1) as cres, \
                 tc.tile_pool(name="cps", bufs=2, space="PSUM") as cps, \
                 tc.tile_pool(name="cev", bufs=3) as cev:
                wo_sb = None
                if "c" in phases:
                    wo_sb = cres.tile([128, HPC, C], F16)
                    qeng = [nc.sync, nc.scalar, nc.gpsimd, nc.sync]
                    for r in range(4):
                        qeng[r].dma_start(
                            out=wo_sb[:, :, r * 512:(r + 1) * 512],
                            in_=wor[:, :, r * 512:(r + 1) * 512])

                # chunk pairs share the wo-block stationary per fc step
                # (second load elided)
                def c_units(pair):
                    units = []
                    ts0 = slice(1024 * pair, 1024 * pair + 512)
                    ts1 = slice(1024 * pair + 512, 1024 * pair + 1024)
                    for ct in range(C // 128):
                        def unit(ct=ct):
                            ps0 = cps.tile([128, 512], F32, tag="cps0",
                                           name="ps0", bufs=2)
                            ps1 = cps.tile([128, 512], F32, tag="cps1",
                                           name="ps1", bufs=2)
                            for fc in range(HPC):
                                wblk = wo_sb[:, fc, ct * 128:(ct + 1) * 128]
                                nc.tensor.matmul(
                                    ps0[:], wblk, yT_sb[:, fc, ts0],
                                    start=(fc == 0), stop=(fc == HPC - 1))
                                mm1 = nc.tensor.matmul(
                                    ps1[:], wblk, yT_sb[:, fc, ts1],
                                    start=(fc == 0), stop=(fc == HPC - 1))
                                if fc == 0 and ct == 0 and \
                                        shared["last_drain"]:
                                    # fence: tile's subtile RAW tracking
                                    # misses some yT reads vs the head-7
                                    # drain writes; PE program order then
                                    # fences every later c matmul
                                    mm1.ins.add_dependency(
                                        shared["last_drain"],
                                        bass_rust.DependencyInfo.SYNC_ONLY)
                            ev = cev.tile([128, 1024], F16, tag="cev")
                            nc.scalar.copy(ev[:, 0:512], ps0[:])
                            nc.scalar.copy(ev[:, 512:1024], ps1[:])
                            nc.sync.dma_start(
                                out=outT[ct * 128:(ct + 1) * 128, ts0],
                                in_=ev[:, 0:512])
                            nc.scalar.dma_start(
                                out=outT[ct * 128:(ct + 1) * 128, ts1],
                                in_=ev[:, 512:1024])
                        units.append(unit)
                    return units

                if "b" in phases and "a" in phases and "c" in phases:
                    b7 = b_quanta(HPC - 1, bst, byp, bsm)
                    for q in b7[0] + b7[1]:
                        q()
                    # pair 0 reads yT chunks 0/1 only (drained above);
                    # pair 1 reads chunks 2/3 so it runs after b7 ends
                    merge_emit(c_units(0), b7[2] + b7[3])
                    for u in c_units(1):
                        u()
                elif "b" in phases and "a" in phases:
                    for q in sum(b_quanta(HPC - 1, bst, byp, bsm), []):
                        q()
                elif "c" in phases:
                    for pair in range(TCH // 2):
                        for u in c_units(pair):
                            u()


def _elide_redundant_ldweights(nc):
    """Drop InstLdweights that reload the stationary operand already
    sitting in the PE array (identical weights AP, no intervening PE
    instruction that clobbers the array).  The cost-model sim treats
    weight loads as free, but hardware pays ~50ns per 128-column load;
    pairing matmuls on the same stationary and eliding the second load
    recovers that time.  Deps of a dropped load move onto the next PE
    instruction; deps of other instructions pointing at a dropped load
    are remapped the same way."""
    n_elided = 0
    for fn in nc.m.functions:
        remap = {}
        for blk in fn.blocks:
            cur_sig = None
            keep = []
            pending = []          # (dep_name, DependencyInfo) from drops
            unresolved = []       # dropped names awaiting a successor
            for inst in blk.instructions:
                t = type(inst).__name__
                if t == "InstLdweights":
                    sig = (str(inst.ins[0]),
                           str(getattr(inst, "perf_mode", None)),
                           bool(getattr(inst, "is_transpose", False) or False),
                           str(getattr(inst, "tile_position", None)))
                    if sig == cur_sig:
                        for d in inst.sync_dependency_names():
                            pending.append((d, inst.get_dependency_info(d)))
                        for d in inst.nosync_dependency_names():
                            pending.append((d, inst.get_dependency_info(d)))
                        unresolved.append(inst.name)
                        n_elided += 1
                        continue
                    cur_sig = sig
                elif t == "InstMatmult":
                    if getattr(inst, "is_transpose", False) or inst.ldweights:
                        cur_sig = None
                if getattr(inst, "engine", None) == mybir.EngineType.PE and \
                        t in ("InstMatmult", "InstLdweights"):
                    if pending:
                        have = set(inst.sync_dependency_names()) | \
                            set(inst.nosync_dependency_names())
                        for d, info in pending:
                            if d not in have and d != inst.name:
                                inst.add_dependency(d, info)
                                have.add(d)
                        pending = []
                    for name in unresolved:
                        remap[name] = inst.name
                    unresolved = []
                keep.append(inst)
            assert not pending and not unresolved, \
                "dropped Ldweights with no PE successor in block"
            blk.instructions = keep
        if remap:
            for blk in fn.blocks:
                for inst in blk.instructions:
                    deps = set(inst.sync_dependency_names()) | \
                        set(inst.nosync_dependency_names())
                    hits = deps & remap.keys()
                    if not hits:
                        continue
                    m = {}
                    for d in hits:
                        tgt = remap[d]
                        if tgt == inst.name or tgt in deps:
                            inst.try_remove_dependency(d)
                        else:
                            m[d] = tgt
                    if m:
                        inst.remap_dependency_names(m)
    return n_elided


_CACHE = {}


def _get_program():
    if "nc" not in _CACHE:
        _CACHE["nc"] = _build_program()
    return _CACHE["nc"]


def _make_in_maps(x, Wqkv, Wout):
    x = np.asarray(x, dtype=np.float32)
    Wqkv = np.asarray(Wqkv, dtype=np.float32)
    Wout = np.asarray(Wout, dtype=np.float32)
    in_maps = []
    for core in range(NCORES):
        b, g = core // 2, core % 2
        fs = slice(g * F, (g + 1) * F)
        wq = Wqkv[:, fs].astype(np.float16)
        wk = Wqkv[:, C:][:, fs].astype(np.float16)
        wv = Wqkv[:, 2 * C:][:, fs].astype(np.float16)
        wo = Wout[fs, :].astype(np.float16)
        in_maps.append({
            "xT": np.ascontiguousarray(
                x[b].T.astype(np.float16)
                .reshape(CCH, 128, T).transpose(1, 0, 2)),
            "wq": np.ascontiguousarray(
                wq.reshape(CCH, 128, HPC, 128).transpose(1, 2, 0, 3)),
            "wk": np.ascontiguousarray(
                wk.reshape(CCH, 128, HPC, 128).transpose(1, 2, 0, 3)),
            "wv": np.ascontiguousarray(
                wv.reshape(CCH, 128, F).transpose(1, 0, 2)),
            "wout": np.ascontiguousarray(
                wo.reshape(HPC, 128, C).transpose(1, 0, 2)),
        })
    return in_maps


def run_sharded(x, Wqkv, Wout, trace=False):
    """Run the SPMD program; returns (out [B,T,C], BassKernelResults)."""
    nc = _get_program()
    res = run_bass_kernel_spmd(
        nc, _make_in_maps(x, Wqkv, Wout), list(range(NCORES)), trace=trace)
    out = np.empty((B, T, C), dtype=np.float32)
    for b in range(B):
        acc = (res.results[2 * b]["outT"].astype(np.float32)
               + res.results[2 * b + 1]["outT"].astype(np.float32))
        out[b] = acc.T
    return out, res


def kernel(x, Wqkv, Wout):
    out, _ = run_sharded(x, Wqkv, Wout, trace=False)
    return out



# revision 25
# speedup vs baseline: 1.0664x; 1.0664x over previous
"""Trainium2 Bass kernel for nn_Attention_62414464746139.

Full causal attention layer: QKV projection + RoPE + causal softmax
attention + output projection.  B=4, T=2048, C=2048, H=16, D=128, f32.

Sharding over 8 NeuronCores: core c handles batch b = c//2 and head
group g = c%2 (8 of the 16 heads).  Each core computes its heads' QKV
projection, attention, and a *partial* output projection (its heads'
rows of Wout); the host sums the two head-group partials per batch.

Fully SBUF-resident dataflow (no DRAM scratch):
  - all weights/activations host-prepped into partition-major layouts so
    every DMA moves >=1KB contiguous lines per partition
  - v computed first (xT stationary, Wv moving), stored [t, f] in SBUF
  - per head h: q,k projected into [d, t] SBUF tiles; RoPE applied on
    the full [128, T] tile (rotate-half = SBUF->SBUF half-swap DMA with
    the sign folded into the +-sin table; 3 full-width DVE ops)
  - attention per head: ST[t2, t1] = kT.T @ qT in 3-block PSUM groups
    [128, 1536]; ONE exp activation per group (with a 2^-12 bias folded
    in so fp16 row sums cannot overflow; the scale cancels in the
    normalize); softmax denominator via DVE accumulation of e into esum
    [128, 512] + ONE ones-matmul per chunk (partition reduce);
    yT[d, t1] = v.T @ e.  Scores/y matmuls and esum adds are narrowed
    to the exact causal column range; the masked-out e region holds
    exp of stale PSUM that no consumer reads (no memsets).
  - emission interleaves head h-1's attention into head h's projection
    so ACT/DVE/GPSIMD work hides under projection matmuls
  - out projection reads yT from SBUF; outT written as fp16 (host sums
    the two head-group partials in f32).

Matmul operands are float16 (full PE rate); PSUM accumulation fp32.
Softmax is computed without max subtraction: scores*scale are O(10), so
exp stays within fp16 range after the 2^-12 bias.
"""

import math

import numpy as np

import bass_rust

import concourse.bacc as bacc
import concourse.bass as bass
import concourse.mybir as mybir
import concourse.tile as tile
from concourse.bass_utils import run_bass_kernel_spmd

B, T, C = 4, 2048, 2048
H, D = 16, 128
HPC = 8            # heads per core
F = HPC * D        # 1024: per-core feature width
NCORES = 8
THETA = 10000.0
SCALE = 1.0 / math.sqrt(D)
NEG = -1.0e30
LOG_ALPHA = -12.0 * math.log(2.0)   # exp bias: e' = 2^-12 * exp(s*scale)

F32 = mybir.dt.float32
F16 = mybir.dt.float16

TCH = T // 512     # 4  t-chunks of 512
CCH = C // 128     # 16 c-chunks of 128
TT = T // 128      # 16 t-tiles of 128


def _rope_tables():
    inv_freq = 1.0 / (THETA ** (np.arange(0, D, 2, dtype=np.float32) / D))
    pos = np.arange(T, dtype=np.float32)
    freqs = np.outer(pos, inv_freq).astype(np.float32)          # [T, D/2]
    emb = np.concatenate([freqs, freqs], axis=-1)               # [T, D]
    cosT = np.cos(emb).T.astype(np.float16).copy()              # [D, T]
    sinT = np.sin(emb).T.astype(np.float16).copy()
    # rotate-half is a plain half-swap if the sign lives in the table:
    # o = raw*cos + swap(raw)*sinS with sinS[:64] = -sin[:64].
    sinS = sinT.copy()
    sinS[: D // 2] = -sinS[: D // 2]
    return cosT, sinS


def _build_program(phases=("a", "b", "c"), repeat=1):
    nc = bacc.Bacc("TRN2", target_bir_lowering=False, debug=False)

    xTr = nc.dram_tensor("xT", [128, CCH, T], F16, kind="ExternalInput")
    wqr = nc.dram_tensor("wq", [128, HPC, CCH, 128], F16,
                         kind="ExternalInput")
    wkr = nc.dram_tensor("wk", [128, HPC, CCH, 128], F16,
                         kind="ExternalInput")
    wvr = nc.dram_tensor("wv", [128, CCH, F], F16, kind="ExternalInput")
    wor = nc.dram_tensor("wout", [128, HPC, C], F16, kind="ExternalInput")
    outT = nc.dram_tensor("outT", [C, T], F16, kind="ExternalOutput")

    cosT_np, sinS_np = _rope_tables()
    r, c = np.arange(128)[:, None], np.arange(128)[None, :]
    trimask_np = np.where(r <= c, 0.0, NEG).astype(np.float32)

    cosT_d = nc.inline_tensor(cosT_np, name="cosT")
    sinS_d = nc.inline_tensor(sinS_np, name="sinS")
    trimask_d = nc.inline_tensor(trimask_np, name="trimask")
    bias_d = nc.inline_tensor(
        np.full((128, 1), LOG_ALPHA, dtype=np.float32), name="ebias")
    ones_d = nc.inline_tensor(np.ones((128, 128), dtype=np.float16),
                              name="onesm")

    with tile.TileContext(nc) as tc, \
         nc.allow_low_precision(reason="fp16 softmax row sums, 2^-12 biased"):
        with tc.tile_pool(name="consts", bufs=1) as consts:
            cosT_sb = consts.tile([D, T], F16)
            sinS_sb = consts.tile([D, T], F16)
            trimask = consts.tile([128, 128], F32)
            ebias = consts.tile([128, 1], F32)
            onesv = consts.tile([128, 128], F16)
            nc.scalar.dma_start(out=cosT_sb[:], in_=cosT_d[:])
            nc.scalar.dma_start(out=sinS_sb[:], in_=sinS_d[:])
            nc.scalar.dma_start(out=trimask[:], in_=trimask_d[:])
            nc.scalar.dma_start(out=ebias[:], in_=bias_d[:])
            nc.scalar.dma_start(out=onesv[:], in_=ones_d[:])

            def run_phases():
                _emit_iteration(nc, tc, phases,
                                xTr, wqr, wkr, wvr, wor, outT,
                                cosT_sb, sinS_sb, trimask, ebias, onesv)

            if repeat == 1:
                run_phases()
            else:
                with tc.For_i(0, repeat, 1):
                    run_phases()
    if not globals().get('_DISABLE_ELIDE'):
        _elide_redundant_ldweights(nc)
    nc.finalize()
    return nc


def _emit_iteration(nc, tc, phases, xTr, wqr, wkr, wvr, wor, outT,
                    cosT_sb, sinS_sb, trimask, ebias, onesv):
    with tc.tile_pool(name="qk", bufs=2) as qk, \
         tc.tile_pool(name="vpool", bufs=1) as vpool, \
         tc.tile_pool(name="ypool", bufs=1) as ypool, \
         tc.tile_pool(name="epool", bufs=2) as epool, \
         tc.tile_pool(name="drain", bufs=2) as drain:

        v_sb = vpool.tile([128, TT, F], F16)      # [t, tt, f]
        yT_sb = ypool.tile([128, HPC, T], F16)    # [d, head, t]
        head_tiles = {}
        shared = {"last_drain": None, "drains": {}}

        # ---------------- attention quanta for one head ---------------
        # Per t1-chunk j (512 cols), t2 blocks 0..4(j+1)-1 are processed
        # in groups of <=3 (one 3-bank PSUM score tile + one exp each).
        # Diagonal blocks (i >= 4j) narrow the scores/y matmuls and the
        # esum adds to cols >= c0; the masked e region is stale garbage
        # that no consumer ever reads (no memsets needed).
        # Softmax denominator: e blocks are accumulated across k-blocks
        # on the DVE into esum [t2=128, t1=512]; ONE ones-matmul per
        # chunk partition-reduces esum (replaces the per-block PE sums
        # matmuls of the previous version).
        def b_quanta(h, bst, byp, bsm):
            qTh, kTh = head_tiles[h]

            # one factory call per chunk so every closure gets its own
            # scope (a plain `for j` loop would rebind the inner defs and
            # leave all quanta pointing at the last chunk's state)
            def make_chunk(j):
                quanta = []
                nblk = 4 * (j + 1)
                groups = [list(range(s, min(s + 2, nblk)))
                          for s in range(0, nblk, 2)]
                state = {}

                def c0_of(i):
                    return max(0, 128 * i - 512 * j)

                def start_chunk():
                    state["yps"] = byp.tile([128, 512], F32, tag="yps",
                                            name="yps")
                    state["esum"] = epool.tile([128, 512], F16, tag="esum",
                                               name="esum")

                def group(blocks):
                    stile = bst.tile([128, 1024], F32, tag="st")
                    e = epool.tile([128, 1024], F16, tag="e")
                    for b, i in enumerate(blocks):
                        c0 = c0_of(i)
                        nc.tensor.matmul(
                            stile[:, b * 512 + c0:(b + 1) * 512],
                            kTh[:, i * 128:(i + 1) * 128],
                            qTh[:, j * 512 + c0:(j + 1) * 512],
                            start=True, stop=True)
                        if i >= 4 * j:
                            nc.vector.tensor_add(
                                stile[:, b * 512 + c0:b * 512 + c0 + 128],
                                stile[:, b * 512 + c0:b * 512 + c0 + 128],
                                trimask[:])
                    # exp over the full group width: cols [b*512, b*512+c0)
                    # of diagonal blocks are exp of stale PSUM, but every
                    # downstream read (esum add, y matmul) narrows past c0.
                    w = 512 * len(blocks)
                    nc.scalar.activation(
                        e[:, 0:w], stile[:, 0:w],
                        mybir.ActivationFunctionType.Exp,
                        bias=ebias[:], scale=SCALE)
                    esum = state["esum"]
                    for b, i in enumerate(blocks):
                        c0 = c0_of(i)
                        if i == 0:
                            nc.vector.tensor_copy(
                                esum[:], e[:, 0:512])
                        else:
                            nc.vector.tensor_add(
                                esum[:, c0:], esum[:, c0:],
                                e[:, b * 512 + c0:(b + 1) * 512])
                    state["e"] = e
                    state["eblocks"] = blocks

                def ymm():
                    e = state["e"]
                    for b, i in enumerate(state["eblocks"]):
                        c0 = c0_of(i)
                        mm = nc.tensor.matmul(
                            state["yps"][:, c0:],
                            v_sb[:, i, h * 128:(h + 1) * 128],
                            e[:, b * 512 + c0:(b + 1) * 512],
                            start=(i == 0), stop=(i == nblk - 1),
                            skip_group_check=True)
                        if i == 0 and shared["last_drain"]:
                            # WAR: byp is single-buffered and tile does
                            # not track the drain's PSUM read
                            mm.ins.add_dependency(
                                shared["last_drain"],
                                bass_rust.DependencyInfo.SYNC_ONLY)

                def drain_chunk():
                    sums = byp.tile([128, 512], F32, tag="yps",
                                    name="sums")
                    nc.tensor.matmul(sums[:], onesv[:], state["esum"][:],
                                     start=True, stop=True)
                    # sums >= the diagonal term 2^-12*exp(|q|^2*scale),
                    # which is far above fp16/f32 underflow: no clamp.
                    recip = drain.tile([128, 512], F32, tag="recip",
                                       bufs=1)
                    nc.vector.reciprocal(recip[:], sums[:])
                    mul = nc.vector.tensor_mul(
                        yT_sb[:, h, j * 512:(j + 1) * 512],
                        state["yps"][:], recip[:])
                    shared["last_drain"] = mul.ins.name
                    shared["drains"][(h, j)] = mul.ins.name

                for g in range(len(groups)):
                    def q_g(g=g):
                        if g == 0:
                            start_chunk()
                        else:
                            ymm()
                        group(groups[g])
                    quanta.append(q_g)

                def q_last():
                    ymm()
                    drain_chunk()
                quanta.append(q_last)
                return quanta

            return [make_chunk(j) for j in range(TCH)]

        # ------------------------- phase A2: v -------------------------
        def emit_a2(xT_sb, wvres, vps):
            wvt = wvres.tile([128, CCH, F], F16)
            for cc4 in range(4):
                nc.gpsimd.dma_start(
                    out=wvt[:, cc4 * 4:(cc4 + 1) * 4, 0:512],
                    in_=wvr[:, cc4 * 4:(cc4 + 1) * 4, 0:512])
            nc.scalar.dma_start(out=wvt[:, :, 512:1024],
                                in_=wvr[:, :, 512:1024])
            for j in range(2):
                for tt in range(TT):
                    ps = vps.tile([128, 512], F32, tag="vps")
                    for cc in range(CCH):
                        nc.tensor.matmul(
                            ps[:],
                            xT_sb[:, cc, tt * 128:(tt + 1) * 128],
                            wvt[:, cc, j * 512:(j + 1) * 512],
                            start=(cc == 0), stop=(cc == CCH - 1))
                    nc.scalar.copy(
                        v_sb[:, tt, j * 512:(j + 1) * 512], ps[:])

        # ----------------- phase A1 quanta for one head ----------------
        whold = {}

        def load_w(wstream, dram, h, nm):
            wt = wstream.tile([128, CCH, 128], F16, tag="w", name=nm)
            nc.sync.dma_start(out=wt[:], in_=dram[:, h])
            return wt

        def a1_quanta(wstream, xT_sb, aps, aev, h):
            wqh, wkh = whold.pop(h)
            qTh = qk.tile([128, T], F16, tag="qT")
            kTh = qk.tile([128, T], F16, tag="kT")
            head_tiles[h] = (qTh, kTh)

            def make_tensor(idx, wt, dst, dram):
                state = {}
                quanta = []

                def start_tensor():
                    state["rawT"] = aev.tile([128, T], F16, tag="raw",
                                             name="rawT", bufs=2)

                for tc_i in range(TCH):
                    def unit(tc_i=tc_i):
                        if tc_i == 0:
                            start_tensor()
                        ts = slice(tc_i * 512, (tc_i + 1) * 512)
                        ps = aps.tile([128, 512], F32, tag="aps")
                        for cc in range(CCH):
                            nc.tensor.matmul(
                                ps[:], wt[:, cc, :], xT_sb[:, cc, ts],
                                start=(cc == 0), stop=(cc == CCH - 1))
                        nc.scalar.copy(state["rawT"][:, ts], ps[:])
                    quanta.append(unit)

                def finish():
                    # prefetch this tensor's weights for head h+1 (its
                    # predecessor buffer is free now)
                    if h + 1 < HPC:
                        whold.setdefault(h + 1, [None, None])[idx] = \
                            load_w(wstream, dram, h + 1, f"w{idx}_{h + 1}")
                    rawT = state["rawT"]
                    rotT = aev.tile([128, T], F16, tag="rot", bufs=2)
                    nc.gpsimd.dma_start(out=rotT[0:64, :],
                                        in_=rawT[64:128, :])
                    nc.gpsimd.dma_start(out=rotT[64:128, :],
                                        in_=rawT[0:64, :])
                    nc.vector.tensor_mul(dst[:], rawT[:], cosT_sb[:])
                    nc.vector.tensor_mul(rotT[:], rotT[:], sinS_sb[:])
                    nc.vector.tensor_add(dst[:], dst[:], rotT[:])
                quanta.append(finish)
                return quanta

            return (make_tensor(0, wqh, qTh, wqr)
                    + make_tensor(1, wkh, kTh, wkr))

        def merge_emit(a_list, b_list, frac=1.0):
            """Emit two quantum streams proportionally interleaved.
            With frac < 1 the b stream finishes when the a stream is at
            that fraction, leaving an a-only tail that hides the b
            drain latency under projection matmuls."""
            na, nb = len(a_list), len(b_list)
            ia = ib = 0
            while ia < na or ib < nb:
                if (ib < nb and ib * frac * max(na, 1) <=
                        ia * max(nb, 1)) or ia >= na:
                    b_list[ib]()
                    ib += 1
                else:
                    a_list[ia]()
                    ia += 1

        # ------------------------- main schedule -----------------------
        with tc.tile_pool(name="bst", bufs=2, space="PSUM") as bst, \
             tc.tile_pool(name="byp", bufs=2, space="PSUM") as byp:
            bsm = None
            with tc.tile_pool(name="xres", bufs=1) as xres:
                xT_sb = xres.tile([128, CCH, T], F16)
                xsplits = [(0, 128), (128, 512), (512, 1024),
                           (1024, 1536), (1536, 2048)]
                for q, (lo, hi) in enumerate(xsplits):
                    eng = nc.sync if q % 2 == 0 else nc.scalar
                    eng.dma_start(
                        out=xT_sb[:, :, lo:hi],
                        in_=xTr[:, :, lo:hi])

                if "a" in phases:
                  with tc.tile_pool(name="wstream", bufs=2) as wstream:
                    whold[0] = [load_w(wstream, wqr, 0, "wq0"),
                                load_w(wstream, wkr, 0, "wk0")]
                    with tc.tile_pool(name="wvres", bufs=1) as wvres, \
                         tc.tile_pool(name="vps", bufs=2,
                                      space="PSUM") as vps:
                        emit_a2(xT_sb, wvres, vps)

                    with tc.tile_pool(name="aps", bufs=2,
                                      space="PSUM") as aps, \
                         tc.tile_pool(name="aev", bufs=2) as aev:
                        for h in range(HPC):
                            a_list = a1_quanta(wstream, xT_sb, aps, aev, h)
                            b_list = (sum(b_quanta(h - 1, bst, byp, bsm), [])
                                      if ("b" in phases and h > 0) else [])
                            merge_emit(a_list, b_list)

            # tail: last head's attention + output projection
            with tc.tile_pool(name="cres", bufs=1) as cres, \
                 tc.tile_pool(name="cps", bufs=2, space="PSUM") as cps, \
                 tc.tile_pool(name="cev", bufs=3) as cev:
                wo_sb = None
                if "c" in phases:
                    wo_sb = cres.tile([128, HPC, C], F16)
                    qeng = [nc.sync, nc.scalar, nc.gpsimd, nc.sync]
                    for r in range(4):
                        qeng[r].dma_start(
                            out=wo_sb[:, :, r * 512:(r + 1) * 512],
                            in_=wor[:, :, r * 512:(r + 1) * 512])

                def c_units(tc_i):
                    units = []
                    ts = slice(tc_i * 512, (tc_i + 1) * 512)
                    for ct in range(C // 128):
                        def unit(ct=ct, ts=ts, tc_i=tc_i):
                            ps = cps.tile([128, 512], F32, tag="cps")
                            for fc in range(HPC):
                                mm = nc.tensor.matmul(
                                    ps[:],
                                    wo_sb[:, fc, ct * 128:(ct + 1) * 128],
                                    yT_sb[:, fc, ts],
                                    start=(fc == 0), stop=(fc == HPC - 1))
                                if fc == 0 and ct == 0:
                                    # fence: tile's subtile RAW tracking
                                    # misses some yT reads vs the head-7
                                    # drain writes; PE program order then
                                    # fences every later c matmul
                                    d = shared["drains"].get(
                                        (HPC - 1, tc_i))
                                    if d:
                                        mm.ins.add_dependency(
                                            d,
                                            bass_rust.DependencyInfo
                                            .SYNC_ONLY)
                            ev = cev.tile([128, 512], F16, tag="cev")
                            nc.scalar.copy(ev[:], ps[:])
                            nc.sync.dma_start(
                                out=outT[ct * 128:(ct + 1) * 128, ts],
                                in_=ev[:])
                        units.append(unit)
                    return units

                if "b" in phases and "a" in phases and "c" in phases:
                    b7 = b_quanta(HPC - 1, bst, byp, bsm)
                    for q in b7[0]:
                        q()
                    for j in range(TCH - 1):
                        merge_emit(c_units(j), b7[j + 1])
                    for u in c_units(TCH - 1):
                        u()
                elif "b" in phases and "a" in phases:
                    for q in sum(b_quanta(HPC - 1, bst, byp, bsm), []):
                        q()
                elif "c" in phases:
                    for tc_i in range(TCH):
                        for u in c_units(tc_i):
                            u()


def _elide_redundant_ldweights(nc):
    """Drop InstLdweights that reload the stationary operand already
    sitting in the PE array (identical weights AP, no intervening PE
    instruction that clobbers the array).  The cost-model sim treats
    weight loads as free, but hardware pays ~50ns per 128-column load;
    pairing matmuls on the same stationary and eliding the second load
    recovers that time.  Deps of a dropped load move onto the next PE
    instruction; deps of other instructions pointing at a dropped load
    are remapped the same way."""
    n_elided = 0
    for fn in nc.m.functions:
        remap = {}
        for blk in fn.blocks:
            cur_sig = None
            keep = []
            pending = []          # (dep_name, DependencyInfo) from drops
            unresolved = []       # dropped names awaiting a successor
            for inst in blk.instructions:
                t = type(inst).__name__
                if t == "InstLdweights":
                    sig = (str(inst.ins[0]),
                           str(getattr(inst, "perf_mode", None)),
                           bool(getattr(inst, "is_transpose", False) or False),
                           str(getattr(inst, "tile_position", None)))
                    if sig == cur_sig:
                        for d in inst.sync_dependency_names():
                            pending.append((d, inst.get_dependency_info(d)))
                        for d in inst.nosync_dependency_names():
                            pending.append((d, inst.get_dependency_info(d)))
                        unresolved.append(inst.name)
                        n_elided += 1
                        continue
                    cur_sig = sig
                elif t == "InstMatmult":
                    if getattr(inst, "is_transpose", False) or inst.ldweights:
                        cur_sig = None
                if getattr(inst, "engine", None) == mybir.EngineType.PE and \
                        t in ("InstMatmult", "InstLdweights"):
                    if pending:
                        have = set(inst.sync_dependency_names()) | \
                            set(inst.nosync_dependency_names())
                        for d, info in pending:
                            if d not in have and d != inst.name:
                                inst.add_dependency(d, info)
                                have.add(d)
                        pending = []
                    for name in unresolved:
                        remap[name] = inst.name
                    unresolved = []
                keep.append(inst)
            assert not pending and not unresolved, \
                "dropped Ldweights with no PE successor in block"
            blk.instructions = keep
        if remap:
            for blk in fn.blocks:
                for inst in blk.instructions:
                    deps = set(inst.sync_dependency_names()) | \
                        set(inst.nosync_dependency_names())
                    hits = deps & remap.keys()
                    if not hits:
                        continue
                    m = {}
                    for d in hits:
                        tgt = remap[d]
                        if tgt == inst.name or tgt in deps:
                            inst.try_remove_dependency(d)
                        else:
                            m[d] = tgt
                    if m:
                        inst.remap_dependency_names(m)
    return n_elided


_CACHE = {}


def _get_program():
    if "nc" not in _CACHE:
        _CACHE["nc"] = _build_program()
    return _CACHE["nc"]


def _make_in_maps(x, Wqkv, Wout):
    x = np.asarray(x, dtype=np.float32)
    Wqkv = np.asarray(Wqkv, dtype=np.float32)
    Wout = np.asarray(Wout, dtype=np.float32)
    in_maps = []
    for core in range(NCORES):
        b, g = core // 2, core % 2
        fs = slice(g * F, (g + 1) * F)
        wq = Wqkv[:, fs].astype(np.float16)
        wk = Wqkv[:, C:][:, fs].astype(np.float16)
        wv = Wqkv[:, 2 * C:][:, fs].astype(np.float16)
        wo = Wout[fs, :].astype(np.float16)
        in_maps.append({
            "xT": np.ascontiguousarray(
                x[b].T.astype(np.float16)
                .reshape(CCH, 128, T).transpose(1, 0, 2)),
            "wq": np.ascontiguousarray(
                wq.reshape(CCH, 128, HPC, 128).transpose(1, 2, 0, 3)),
            "wk": np.ascontiguousarray(
                wk.reshape(CCH, 128, HPC, 128).transpose(1, 2, 0, 3)),
            "wv": np.ascontiguousarray(
                wv.reshape(CCH, 128, F).transpose(1, 0, 2)),
            "wout": np.ascontiguousarray(
                wo.reshape(HPC, 128, C).transpose(1, 0, 2)),
        })
    return in_maps


def run_sharded(x, Wqkv, Wout, trace=False):
    """Run the SPMD program; returns (out [B,T,C], BassKernelResults)."""
    nc = _get_program()
    res = run_bass_kernel_spmd(
        nc, _make_in_maps(x, Wqkv, Wout), list(range(NCORES)), trace=trace)
    out = np.empty((B, T, C), dtype=np.float32)
    for b in range(B):
        acc = (res.results[2 * b]["outT"].astype(np.float32)
               + res.results[2 * b + 1]["outT"].astype(np.float32))
        out[b] = acc.T
    return out, res


def kernel(x, Wqkv, Wout):
    out, _ = run_sharded(x, Wqkv, Wout, trace=False)
    return out



# revision 31
# speedup vs baseline: 1.0669x; 1.0004x over previous
"""Trainium2 Bass kernel for nn_Attention_62414464746139.

Full causal attention layer: QKV projection + RoPE + causal softmax
attention + output projection.  B=4, T=2048, C=2048, H=16, D=128, f32.

Sharding over 8 NeuronCores: core c handles batch b = c//2 and head
group g = c%2 (8 of the 16 heads).  Each core computes its heads' QKV
projection, attention, and a *partial* output projection (its heads'
rows of Wout); the host sums the two head-group partials per batch.

Fully SBUF-resident dataflow (no DRAM scratch):
  - all weights/activations host-prepped into partition-major layouts so
    every DMA moves >=1KB contiguous lines per partition
  - v computed first (xT stationary, Wv moving), stored [t, f] in SBUF
  - per head h: q,k projected into [d, t] SBUF tiles; RoPE applied on
    the full [128, T] tile (rotate-half = SBUF->SBUF half-swap DMA with
    the sign folded into the +-sin table; 3 full-width DVE ops)
  - attention per head: ST[t2, t1] = kT.T @ qT in 2-block PSUM groups
    [128, 1024], double-buffered so adjacent groups pipeline through
    the exp; ONE exp activation per group (with a 2^-12 bias folded
    in so fp16 row sums cannot overflow; the scale cancels in the
    normalize); softmax denominator via DVE accumulation of e into esum
    [128, 512] + ONE ones-matmul per chunk (partition reduce; its PSUM
    tile shares the byp pool);
    yT[d, t1] = v.T @ e.  Scores/y matmuls and esum adds are narrowed
    to the exact causal column range; the masked-out e region holds
    exp of stale PSUM that no consumer reads (no memsets).  Tile does
    not WAR/RAW-track the PSUM drain reads and some yT subtile reads,
    so explicit fence deps order each yps generation after the previous
    drain and the c-phase after the head-7 drains.
  - emission interleaves head h-1's attention into head h's projection
    so ACT/DVE/GPSIMD work hides under projection matmuls
  - out projection reads yT from SBUF; outT written as fp16 (host sums
    the two head-group partials in f32).

Matmul operands are float16 (full PE rate); PSUM accumulation fp32.
Softmax is computed without max subtraction: scores*scale are O(10), so
exp stays within fp16 range after the 2^-12 bias.
"""

import math

import numpy as np

import bass_rust

import concourse.bacc as bacc
import concourse.bass as bass
import concourse.mybir as mybir
import concourse.tile as tile
from concourse.bass_utils import run_bass_kernel_spmd

B, T, C = 4, 2048, 2048
H, D = 16, 128
HPC = 8            # heads per core
F = HPC * D        # 1024: per-core feature width
NCORES = 8
THETA = 10000.0
SCALE = 1.0 / math.sqrt(D)
NEG = -1.0e30
LOG_ALPHA = -12.0 * math.log(2.0)   # exp bias: e' = 2^-12 * exp(s*scale)

F32 = mybir.dt.float32
F16 = mybir.dt.float16

TCH = T // 512     # 4  t-chunks of 512
CCH = C // 128     # 16 c-chunks of 128
TT = T // 128      # 16 t-tiles of 128


def _rope_tables():
    inv_freq = 1.0 / (THETA ** (np.arange(0, D, 2, dtype=np.float32) / D))
    pos = np.arange(T, dtype=np.float32)
    freqs = np.outer(pos, inv_freq).astype(np.float32)          # [T, D/2]
    emb = np.concatenate([freqs, freqs], axis=-1)               # [T, D]
    cosT = np.cos(emb).T.astype(np.float16).copy()              # [D, T]
    sinT = np.sin(emb).T.astype(np.float16).copy()
    # rotate-half is a plain half-swap if the sign lives in the table:
    # o = raw*cos + swap(raw)*sinS with sinS[:64] = -sin[:64].
    sinS = sinT.copy()
    sinS[: D // 2] = -sinS[: D // 2]
    return cosT, sinS


def _build_program(phases=("a", "b", "c"), repeat=1):
    nc = bacc.Bacc("TRN2", target_bir_lowering=False, debug=False)

    xTr = nc.dram_tensor("xT", [128, CCH, T], F16, kind="ExternalInput")
    wqr = nc.dram_tensor("wq", [128, HPC, CCH, 128], F16,
                         kind="ExternalInput")
    wkr = nc.dram_tensor("wk", [128, HPC, CCH, 128], F16,
                         kind="ExternalInput")
    wvr = nc.dram_tensor("wv", [128, CCH, F], F16, kind="ExternalInput")
    wor = nc.dram_tensor("wout", [128, HPC, C], F16, kind="ExternalInput")
    outT = nc.dram_tensor("outT", [C, T], F16, kind="ExternalOutput")

    cosT_np, sinS_np = _rope_tables()
    r, c = np.arange(128)[:, None], np.arange(128)[None, :]
    trimask_np = np.where(r <= c, 0.0, NEG).astype(np.float32)

    cosT_d = nc.inline_tensor(cosT_np, name="cosT")
    sinS_d = nc.inline_tensor(sinS_np, name="sinS")
    trimask_d = nc.inline_tensor(trimask_np, name="trimask")
    bias_d = nc.inline_tensor(
        np.full((128, 1), LOG_ALPHA, dtype=np.float32), name="ebias")
    ones_d = nc.inline_tensor(np.ones((128, 128), dtype=np.float16),
                              name="onesm")

    with tile.TileContext(nc) as tc, \
         nc.allow_low_precision(reason="fp16 softmax row sums, 2^-12 biased"):
        with tc.tile_pool(name="consts", bufs=1) as consts:
            cosT_sb = consts.tile([D, T], F16)
            sinS_sb = consts.tile([D, T], F16)
            trimask = consts.tile([128, 128], F32)
            ebias = consts.tile([128, 1], F32)
            onesv = consts.tile([128, 128], F16)
            nc.scalar.dma_start(out=cosT_sb[:], in_=cosT_d[:])
            nc.scalar.dma_start(out=sinS_sb[:], in_=sinS_d[:])
            nc.scalar.dma_start(out=trimask[:], in_=trimask_d[:])
            nc.scalar.dma_start(out=ebias[:], in_=bias_d[:])
            nc.scalar.dma_start(out=onesv[:], in_=ones_d[:])

            def run_phases():
                _emit_iteration(nc, tc, phases,
                                xTr, wqr, wkr, wvr, wor, outT,
                                cosT_sb, sinS_sb, trimask, ebias, onesv)

            if repeat == 1:
                run_phases()
            else:
                with tc.For_i(0, repeat, 1):
                    run_phases()
    if not globals().get('_DISABLE_ELIDE'):
        _elide_redundant_ldweights(nc)
    nc.finalize()
    return nc


def _emit_iteration(nc, tc, phases, xTr, wqr, wkr, wvr, wor, outT,
                    cosT_sb, sinS_sb, trimask, ebias, onesv):
    with tc.tile_pool(name="qk", bufs=2) as qk, \
         tc.tile_pool(name="vpool", bufs=1) as vpool, \
         tc.tile_pool(name="ypool", bufs=1) as ypool, \
         tc.tile_pool(name="epool", bufs=2) as epool, \
         tc.tile_pool(name="drain", bufs=2) as drain:

        v_sb = vpool.tile([128, TT, F], F16)      # [t, tt, f]
        yT_sb = ypool.tile([128, HPC, T], F16)    # [d, head, t]
        head_tiles = {}
        shared = {"last_drain": None, "drains": {}}

        # ---------------- attention quanta for one head ---------------
        # Per t1-chunk j (512 cols), t2 blocks 0..4(j+1)-1 are processed
        # in groups of <=3 (one 3-bank PSUM score tile + one exp each).
        # Diagonal blocks (i >= 4j) narrow the scores/y matmuls and the
        # esum adds to cols >= c0; the masked e region is stale garbage
        # that no consumer ever reads (no memsets needed).
        # Softmax denominator: e blocks are accumulated across k-blocks
        # on the DVE into esum [t2=128, t1=512]; ONE ones-matmul per
        # chunk partition-reduces esum (replaces the per-block PE sums
        # matmuls of the previous version).
        def b_quanta(h, bst, byp, bsm):
            qTh, kTh = head_tiles[h]

            # one factory call per chunk so every closure gets its own
            # scope (a plain `for j` loop would rebind the inner defs and
            # leave all quanta pointing at the last chunk's state)
            def make_chunk(j):
                quanta = []
                nblk = 4 * (j + 1)
                groups = [list(range(s, min(s + 2, nblk)))
                          for s in range(0, nblk, 2)]
                state = {}

                def c0_of(i):
                    return max(0, 128 * i - 512 * j)

                def start_chunk():
                    state["yps"] = byp.tile([128, 512], F32, tag="yps",
                                            name="yps")
                    state["esum"] = epool.tile([128, 512], F16, tag="esum",
                                               name="esum")

                def group(blocks):
                    stile = bst.tile([128, 1024], F32, tag="st")
                    e = epool.tile([128, 1024], F16, tag="e")
                    for b, i in enumerate(blocks):
                        c0 = c0_of(i)
                        nc.tensor.matmul(
                            stile[:, b * 512 + c0:(b + 1) * 512],
                            kTh[:, i * 128:(i + 1) * 128],
                            qTh[:, j * 512 + c0:(j + 1) * 512],
                            start=True, stop=True)
                        if i >= 4 * j:
                            nc.vector.tensor_add(
                                stile[:, b * 512 + c0:b * 512 + c0 + 128],
                                stile[:, b * 512 + c0:b * 512 + c0 + 128],
                                trimask[:])
                    # exp over the full group width: cols [b*512, b*512+c0)
                    # of diagonal blocks are exp of stale PSUM, but every
                    # downstream read (esum add, y matmul) narrows past c0.
                    w = 512 * len(blocks)
                    nc.scalar.activation(
                        e[:, 0:w], stile[:, 0:w],
                        mybir.ActivationFunctionType.Exp,
                        bias=ebias[:], scale=SCALE)
                    esum = state["esum"]
                    for b, i in enumerate(blocks):
                        c0 = c0_of(i)
                        if i == 0:
                            nc.vector.tensor_copy(
                                esum[:], e[:, 0:512])
                        else:
                            nc.vector.tensor_add(
                                esum[:, c0:], esum[:, c0:],
                                e[:, b * 512 + c0:(b + 1) * 512])
                    state["e"] = e
                    state["eblocks"] = blocks

                def ymm():
                    e = state["e"]
                    for b, i in enumerate(state["eblocks"]):
                        c0 = c0_of(i)
                        mm = nc.tensor.matmul(
                            state["yps"][:, c0:],
                            v_sb[:, i, h * 128:(h + 1) * 128],
                            e[:, b * 512 + c0:(b + 1) * 512],
                            start=(i == 0), stop=(i == nblk - 1),
                            skip_group_check=True)
                        if i == 0 and shared["last_drain"]:
                            # WAR: byp is single-buffered and tile does
                            # not track the drain's PSUM read
                            mm.ins.add_dependency(
                                shared["last_drain"],
                                bass_rust.DependencyInfo.SYNC_ONLY)

                def drain_chunk():
                    sums = byp.tile([128, 512], F32, tag="yps",
                                    name="sums")
                    nc.tensor.matmul(sums[:], onesv[:], state["esum"][:],
                                     start=True, stop=True)
                    # sums >= the diagonal term 2^-12*exp(|q|^2*scale),
                    # which is far above fp16/f32 underflow: no clamp.
                    recip = drain.tile([128, 512], F32, tag="recip",
                                       bufs=1)
                    nc.vector.reciprocal(recip[:], sums[:])
                    mul = nc.vector.tensor_mul(
                        yT_sb[:, h, j * 512:(j + 1) * 512],
                        state["yps"][:], recip[:])
                    shared["last_drain"] = mul.ins.name
                    shared["drains"][(h, j)] = mul.ins.name

                for g in range(len(groups)):
                    def q_g(g=g):
                        if g == 0:
                            start_chunk()
                        else:
                            ymm()
                        group(groups[g])
                    quanta.append(q_g)

                def q_last():
                    ymm()
                    drain_chunk()
                quanta.append(q_last)
                return quanta

            return [make_chunk(j) for j in range(TCH)]

        # ------------------------- phase A2: v -------------------------
        def emit_a2(xT_sb, wvres, vps):
            wvt = wvres.tile([128, CCH, F], F16)
            for cc4 in range(4):
                nc.gpsimd.dma_start(
                    out=wvt[:, cc4 * 4:(cc4 + 1) * 4, 0:512],
                    in_=wvr[:, cc4 * 4:(cc4 + 1) * 4, 0:512])
            nc.scalar.dma_start(out=wvt[:, :, 512:1024],
                                in_=wvr[:, :, 512:1024])
            for j in range(2):
                for tt in range(TT):
                    ps = vps.tile([128, 512], F32, tag="vps")
                    for cc in range(CCH):
                        nc.tensor.matmul(
                            ps[:],
                            xT_sb[:, cc, tt * 128:(tt + 1) * 128],
                            wvt[:, cc, j * 512:(j + 1) * 512],
                            start=(cc == 0), stop=(cc == CCH - 1))
                    nc.scalar.copy(
                        v_sb[:, tt, j * 512:(j + 1) * 512], ps[:])

        # ----------------- phase A1 quanta for one head ----------------
        whold = {}

        def load_w(wstream, dram, h, nm):
            wt = wstream.tile([128, CCH, 128], F16, tag="w", name=nm)
            nc.sync.dma_start(out=wt[:], in_=dram[:, h])
            return wt

        def a1_quanta(wstream, xT_sb, aps, aev, h):
            wqh, wkh = whold.pop(h)
            qTh = qk.tile([128, T], F16, tag="qT")
            kTh = qk.tile([128, T], F16, tag="kT")
            head_tiles[h] = (qTh, kTh)

            def make_tensor(idx, wt, dst, dram):
                state = {}
                quanta = []

                def start_tensor():
                    state["rawT"] = aev.tile([128, T], F16, tag="raw",
                                             name="rawT", bufs=2)

                for tc_i in range(TCH):
                    def unit(tc_i=tc_i):
                        if tc_i == 0:
                            start_tensor()
                        ts = slice(tc_i * 512, (tc_i + 1) * 512)
                        ps = aps.tile([128, 512], F32, tag="aps")
                        for cc in range(CCH):
                            nc.tensor.matmul(
                                ps[:], wt[:, cc, :], xT_sb[:, cc, ts],
                                start=(cc == 0), stop=(cc == CCH - 1))
                        nc.scalar.copy(state["rawT"][:, ts], ps[:])
                    quanta.append(unit)

                def finish():
                    # prefetch this tensor's weights for head h+1 (its
                    # predecessor buffer is free now)
                    if h + 1 < HPC:
                        whold.setdefault(h + 1, [None, None])[idx] = \
                            load_w(wstream, dram, h + 1, f"w{idx}_{h + 1}")
                    rawT = state["rawT"]
                    rotT = aev.tile([128, T], F16, tag="rot", bufs=2)
                    nc.gpsimd.dma_start(out=rotT[0:64, :],
                                        in_=rawT[64:128, :])
                    nc.gpsimd.dma_start(out=rotT[64:128, :],
                                        in_=rawT[0:64, :])
                    nc.vector.tensor_mul(dst[:], rawT[:], cosT_sb[:])
                    nc.vector.tensor_mul(rotT[:], rotT[:], sinS_sb[:])
                    nc.vector.tensor_add(dst[:], dst[:], rotT[:])
                quanta.append(finish)
                return quanta

            return (make_tensor(0, wqh, qTh, wqr)
                    + make_tensor(1, wkh, kTh, wkr))

        def merge_emit(a_list, b_list, frac=1.0):
            """Emit two quantum streams proportionally interleaved.
            With frac < 1 the b stream finishes when the a stream is at
            that fraction, leaving an a-only tail that hides the b
            drain latency under projection matmuls."""
            na, nb = len(a_list), len(b_list)
            ia = ib = 0
            while ia < na or ib < nb:
                if (ib < nb and ib * frac * max(na, 1) <=
                        ia * max(nb, 1)) or ia >= na:
                    b_list[ib]()
                    ib += 1
                else:
                    a_list[ia]()
                    ia += 1

        # ------------------------- main schedule -----------------------
        with tc.tile_pool(name="bst", bufs=2, space="PSUM") as bst, \
             tc.tile_pool(name="byp", bufs=2, space="PSUM") as byp:
            bsm = None
            with tc.tile_pool(name="xres", bufs=1) as xres:
                xT_sb = xres.tile([128, CCH, T], F16)
                xsplits = [(0, 128), (128, 512), (512, 1024),
                           (1024, 1536), (1536, 2048)]
                for q, (lo, hi) in enumerate(xsplits):
                    eng = nc.sync if q % 2 == 0 else nc.scalar
                    eng.dma_start(
                        out=xT_sb[:, :, lo:hi],
                        in_=xTr[:, :, lo:hi])

                if "a" in phases:
                  with tc.tile_pool(name="wstream", bufs=2) as wstream:
                    whold[0] = [load_w(wstream, wqr, 0, "wq0"),
                                load_w(wstream, wkr, 0, "wk0")]
                    with tc.tile_pool(name="wvres", bufs=1) as wvres, \
                         tc.tile_pool(name="vps", bufs=2,
                                      space="PSUM") as vps:
                        emit_a2(xT_sb, wvres, vps)

                    with tc.tile_pool(name="aps", bufs=2,
                                      space="PSUM") as aps, \
                         tc.tile_pool(name="aev", bufs=2) as aev:
                        for h in range(HPC):
                            a_list = a1_quanta(wstream, xT_sb, aps, aev, h)
                            b_list = (sum(b_quanta(h - 1, bst, byp, bsm), [])
                                      if ("b" in phases and h > 0) else [])
                            merge_emit(a_list, b_list)

            # tail: last head's attention + output projection
            with tc.tile_pool(name="cres", bufs=1) as cres, \
                 tc.tile_pool(name="cps", bufs=2, space="PSUM") as cps, \
                 tc.tile_pool(name="cev", bufs=3) as cev:
                wo_sb = None
                if "c" in phases:
                    wo_sb = cres.tile([128, HPC, C], F16)
                    qeng = [nc.sync, nc.scalar, nc.gpsimd, nc.sync]
                    for r in range(4):
                        qeng[r].dma_start(
                            out=wo_sb[:, :, r * 512:(r + 1) * 512],
                            in_=wor[:, :, r * 512:(r + 1) * 512])

                def c_units(tc_i):
                    units = []
                    ts = slice(tc_i * 512, (tc_i + 1) * 512)
                    for ct in range(C // 128):
                        def unit(ct=ct, ts=ts, tc_i=tc_i):
                            ps = cps.tile([128, 512], F32, tag="cps")
                            for fc in range(HPC):
                                mm = nc.tensor.matmul(
                                    ps[:],
                                    wo_sb[:, fc, ct * 128:(ct + 1) * 128],
                                    yT_sb[:, fc, ts],
                                    start=(fc == 0), stop=(fc == HPC - 1))
                                if fc == 0 and ct == 0:
                                    # fence: tile's subtile RAW tracking
                                    # misses some yT reads vs the head-7
                                    # drain writes; PE program order then
                                    # fences every later c matmul
                                    d = shared["drains"].get(
                                        (HPC - 1, tc_i))
                                    if d:
                                        mm.ins.add_dependency(
                                            d,
                                            bass_rust.DependencyInfo
                                            .SYNC_ONLY)
                            ev = cev.tile([128, 512], F16, tag="cev")
                            nc.scalar.copy(ev[:], ps[:])
                            nc.sync.dma_start(
                                out=outT[ct * 128:(ct + 1) * 128, ts],
                                in_=ev[:])
                        units.append(unit)
                    return units

                if "b" in phases and "a" in phases and "c" in phases:
                    b7 = b_quanta(HPC - 1, bst, byp, bsm)
                    for q in b7[0]:
                        q()
                    for j in range(TCH - 1):
                        merge_emit(c_units(j), b7[j + 1])
                    for u in c_units(TCH - 1):
                        u()
                elif "b" in phases and "a" in phases:
                    for q in sum(b_quanta(HPC - 1, bst, byp, bsm), []):
                        q()
                elif "c" in phases:
                    for tc_i in range(TCH):
                        for u in c_units(tc_i):
                            u()


def _elide_redundant_ldweights(nc):
    """Drop InstLdweights that reload the stationary operand already
    sitting in the PE array (identical weights AP, no intervening PE
    instruction that clobbers the array).  The cost-model sim treats
    weight loads as free, but hardware pays ~50ns per 128-column load;
    pairing matmuls on the same stationary and eliding the second load
    recovers that time.  Deps of a dropped load move onto the next PE
    instruction; deps of other instructions pointing at a dropped load
    are remapped the same way."""
    n_elided = 0
    for fn in nc.m.functions:
        remap = {}
        for blk in fn.blocks:
            cur_sig = None
            keep = []
            pending = []          # (dep_name, DependencyInfo) from drops
            unresolved = []       # dropped names awaiting a successor
            for inst in blk.instructions:
                t = type(inst).__name__
                if t == "InstLdweights":
                    sig = (str(inst.ins[0]),
                           str(getattr(inst, "perf_mode", None)),
                           bool(getattr(inst, "is_transpose", False) or False),
                           str(getattr(inst, "tile_position", None)))
                    if sig == cur_sig:
                        for d in inst.sync_dependency_names():
                            pending.append((d, inst.get_dependency_info(d)))
                        for d in inst.nosync_dependency_names():
                            pending.append((d, inst.get_dependency_info(d)))
                        unresolved.append(inst.name)
                        n_elided += 1
                        continue
                    cur_sig = sig
                elif t == "InstMatmult":
                    if getattr(inst, "is_transpose", False) or inst.ldweights:
                        cur_sig = None
                if getattr(inst, "engine", None) == mybir.EngineType.PE and \
                        t in ("InstMatmult", "InstLdweights"):
                    if pending:
                        have = set(inst.sync_dependency_names()) | \
                            set(inst.nosync_dependency_names())
                        for d, info in pending:
                            if d not in have and d != inst.name:
                                inst.add_dependency(d, info)
                                have.add(d)
                        pending = []
                    for name in unresolved:
                        remap[name] = inst.name
                    unresolved = []
                keep.append(inst)
            assert not pending and not unresolved, \
                "dropped Ldweights with no PE successor in block"
            blk.instructions = keep
        if remap:
            for blk in fn.blocks:
                for inst in blk.instructions:
                    deps = set(inst.sync_dependency_names()) | \
                        set(inst.nosync_dependency_names())
                    hits = deps & remap.keys()
                    if not hits:
                        continue
                    m = {}
                    for d in hits:
                        tgt = remap[d]
                        if tgt == inst.name or tgt in deps:
                            inst.try_remove_dependency(d)
                        else:
                            m[d] = tgt
                    if m:
                        inst.remap_dependency_names(m)
    return n_elided


_CACHE = {}


def _get_program():
    if "nc" not in _CACHE:
        _CACHE["nc"] = _build_program()
    return _CACHE["nc"]


def _make_in_maps(x, Wqkv, Wout):
    x = np.asarray(x, dtype=np.float32)
    Wqkv = np.asarray(Wqkv, dtype=np.float32)
    Wout = np.asarray(Wout, dtype=np.float32)
    in_maps = []
    for core in range(NCORES):
        b, g = core // 2, core % 2
        fs = slice(g * F, (g + 1) * F)
        wq = Wqkv[:, fs].astype(np.float16)
        wk = Wqkv[:, C:][:, fs].astype(np.float16)
        wv = Wqkv[:, 2 * C:][:, fs].astype(np.float16)
        wo = Wout[fs, :].astype(np.float16)
        in_maps.append({
            "xT": np.ascontiguousarray(
                x[b].T.astype(np.float16)
                .reshape(CCH, 128, T).transpose(1, 0, 2)),
            "wq": np.ascontiguousarray(
                wq.reshape(CCH, 128, HPC, 128).transpose(1, 2, 0, 3)),
            "wk": np.ascontiguousarray(
                wk.reshape(CCH, 128, HPC, 128).transpose(1, 2, 0, 3)),
            "wv": np.ascontiguousarray(
                wv.reshape(CCH, 128, F).transpose(1, 0, 2)),
            "wout": np.ascontiguousarray(
                wo.reshape(HPC, 128, C).transpose(1, 0, 2)),
        })
    return in_maps


def run_sharded(x, Wqkv, Wout, trace=False):
    """Run the SPMD program; returns (out [B,T,C], BassKernelResults)."""
    nc = _get_program()
    res = run_bass_kernel_spmd(
        nc, _make_in_maps(x, Wqkv, Wout), list(range(NCORES)), trace=trace)
    out = np.empty((B, T, C), dtype=np.float32)
    for b in range(B):
        acc = (res.results[2 * b]["outT"].astype(np.float32)
               + res.results[2 * b + 1]["outT"].astype(np.float32))
        out[b] = acc.T
    return out, res


def kernel(x, Wqkv, Wout):
    out, _ = run_sharded(x, Wqkv, Wout, trace=False)
    return out



# revision 40
# speedup vs baseline: 1.1280x; 1.0573x over previous
"""Trainium2 Bass kernel for nn_Attention_62414464746139.

Full causal attention layer: QKV projection + RoPE + causal softmax
attention + output projection.  B=4, T=2048, C=2048, H=16, D=128, f32.

Sharding over 8 NeuronCores: core c handles batch b = c//2 and head
group g = c%2 (8 of the 16 heads).  Each core computes its heads' QKV
projection, attention, and a *partial* output projection (its heads'
rows of Wout); the host sums the two head-group partials per batch.

Fully SBUF-resident dataflow (no DRAM scratch):
  - all weights/activations host-prepped into partition-major layouts so
    every DMA moves >=1KB contiguous lines per partition
  - v computed first (xT stationary, Wv moving), stored [t, f] in SBUF
  - per head h: q,k projected into [d, t] SBUF tiles; RoPE applied on
    the full [128, T] tile (rotate-half = SBUF->SBUF half-swap DMA with
    the sign folded into the +-sin table; 3 full-width DVE ops)
  - attention per head: ST[t2, t1] = kT.T @ qT in 2-block PSUM groups
    [128, 1024], double-buffered so adjacent groups pipeline through
    the exp; ONE exp activation per group (with a 2^-12 bias folded
    in so fp16 row sums cannot overflow; the scale cancels in the
    normalize); softmax denominator via DVE accumulation of e into esum
    [128, 512] + ONE ones-matmul per chunk (partition reduce; its PSUM
    tile shares the byp pool);
    yT[d, t1] = v.T @ e.  Scores/y matmuls and esum adds are narrowed
    to the exact causal column range; the masked-out e region holds
    exp of stale PSUM that no consumer reads (no memsets).  Tile does
    not WAR/RAW-track the PSUM drain reads and some yT subtile reads,
    so explicit fence deps order each yps generation after the previous
    drain and the c-phase after the head-7 drains.
  - emission interleaves head h-1's attention into head h's projection
    so ACT/DVE/GPSIMD work hides under projection matmuls
  - out projection reads yT from SBUF; outT written as fp16 (host sums
    the two head-group partials in f32).

Matmul operands are float16 (full PE rate); PSUM accumulation fp32.
Softmax is computed without max subtraction: scores*scale are O(10), so
exp stays within fp16 range after the 2^-12 bias.
"""

import math

import numpy as np

import bass_rust

import concourse.bacc as bacc
import concourse.bass as bass
import concourse.mybir as mybir
import concourse.tile as tile
from concourse.bass_utils import run_bass_kernel_spmd

B, T, C = 4, 2048, 2048
H, D = 16, 128
HPC = 8            # heads per core
F = HPC * D        # 1024: per-core feature width
NCORES = 8
THETA = 10000.0
SCALE = 1.0 / math.sqrt(D)
NEG = -1.0e30
LOG_ALPHA = -12.0 * math.log(2.0)   # exp bias: e' = 2^-12 * exp(s*scale)

F32 = mybir.dt.float32
F16 = mybir.dt.float16

TCH = T // 512     # 4  t-chunks of 512
CCH = C // 128     # 16 c-chunks of 128
TT = T // 128      # 16 t-tiles of 128


def _rope_tables():
    inv_freq = 1.0 / (THETA ** (np.arange(0, D, 2, dtype=np.float32) / D))
    pos = np.arange(T, dtype=np.float32)
    freqs = np.outer(pos, inv_freq).astype(np.float32)          # [T, D/2]
    emb = np.concatenate([freqs, freqs], axis=-1)               # [T, D]
    cosT = np.cos(emb).T.astype(np.float16).copy()              # [D, T]
    sinT = np.sin(emb).T.astype(np.float16).copy()
    # rotate-half is a plain half-swap if the sign lives in the table:
    # o = raw*cos + swap(raw)*sinS with sinS[:64] = -sin[:64].
    sinS = sinT.copy()
    sinS[: D // 2] = -sinS[: D // 2]
    return cosT, sinS


def _build_program(phases=("a", "b", "c"), repeat=1):
    nc = bacc.Bacc("TRN2", target_bir_lowering=False, debug=False)

    xTr = nc.dram_tensor("xT", [128, CCH, T], F16, kind="ExternalInput")
    wqr = nc.dram_tensor("wq", [128, HPC, CCH, 128], F16,
                         kind="ExternalInput")
    wkr = nc.dram_tensor("wk", [128, HPC, CCH, 128], F16,
                         kind="ExternalInput")
    wvr = nc.dram_tensor("wv", [128, CCH, F], F16, kind="ExternalInput")
    wor = nc.dram_tensor("wout", [128, HPC, C], F16, kind="ExternalInput")
    outT = nc.dram_tensor("outT", [C, T], F16, kind="ExternalOutput")

    cosT_np, sinS_np = _rope_tables()
    r, c = np.arange(128)[:, None], np.arange(128)[None, :]
    trimask_np = np.where(r <= c, 0.0, NEG).astype(np.float32)

    cosT_d = nc.inline_tensor(cosT_np, name="cosT")
    sinS_d = nc.inline_tensor(sinS_np, name="sinS")
    trimask_d = nc.inline_tensor(trimask_np, name="trimask")
    bias_d = nc.inline_tensor(
        np.full((128, 1), LOG_ALPHA, dtype=np.float32), name="ebias")
    ones_d = nc.inline_tensor(np.ones((128, 128), dtype=np.float16),
                              name="onesm")

    with tile.TileContext(nc) as tc, \
         nc.allow_low_precision(reason="fp16 softmax row sums, 2^-12 biased"):
        with tc.tile_pool(name="consts", bufs=1) as consts:
            cosT_sb = consts.tile([D, T], F16)
            sinS_sb = consts.tile([D, T], F16)
            trimask = consts.tile([128, 128], F32)
            ebias = consts.tile([128, 1], F32)
            onesv = consts.tile([128, 128], F16)
            nc.scalar.dma_start(out=cosT_sb[:], in_=cosT_d[:])
            nc.scalar.dma_start(out=sinS_sb[:], in_=sinS_d[:])
            nc.scalar.dma_start(out=trimask[:], in_=trimask_d[:])
            nc.scalar.dma_start(out=ebias[:], in_=bias_d[:])
            nc.scalar.dma_start(out=onesv[:], in_=ones_d[:])

            def run_phases():
                _emit_iteration(nc, tc, phases,
                                xTr, wqr, wkr, wvr, wor, outT,
                                cosT_sb, sinS_sb, trimask, ebias, onesv)

            if repeat == 1:
                run_phases()
            else:
                with tc.For_i(0, repeat, 1):
                    run_phases()
    if not globals().get('_DISABLE_ELIDE'):
        _elide_redundant_ldweights(nc)
    nc.finalize()
    return nc


def _emit_iteration(nc, tc, phases, xTr, wqr, wkr, wvr, wor, outT,
                    cosT_sb, sinS_sb, trimask, ebias, onesv):
    with tc.tile_pool(name="qk", bufs=2) as qk, \
         tc.tile_pool(name="vpool", bufs=1) as vpool, \
         tc.tile_pool(name="ypool", bufs=1) as ypool, \
         tc.tile_pool(name="epool", bufs=2) as epool, \
         tc.tile_pool(name="drain", bufs=2) as drain:

        v_sb = vpool.tile([128, TT, F], F16)      # [t, tt, f]
        yT_sb = ypool.tile([128, HPC, T], F16)    # [d, head, t]
        head_tiles = {}
        shared = {"last_drain": None, "drains": {}, "qrope": {}}

        # ---------------- attention quanta for one head ---------------
        # Per t1-chunk j (512 cols), t2 blocks 0..4(j+1)-1 are processed
        # in groups of <=3 (one 3-bank PSUM score tile + one exp each).
        # Diagonal blocks (i >= 4j) narrow the scores/y matmuls and the
        # esum adds to cols >= c0; the masked e region is stale garbage
        # that no consumer ever reads (no memsets needed).
        # Softmax denominator: e blocks are accumulated across k-blocks
        # on the DVE into esum [t2=128, t1=512]; ONE ones-matmul per
        # chunk partition-reduces esum (replaces the per-block PE sums
        # matmuls of the previous version).
        def b_quanta(h, bst, byp, bsm, qfence=None):
            qTh, kTh = head_tiles[h]

            # one factory call per chunk so every closure gets its own
            # scope (a plain `for j` loop would rebind the inner defs and
            # leave all quanta pointing at the last chunk's state)
            def make_chunk(j):
                quanta = []
                nblk = 4 * (j + 1)
                groups = [[s] for s in range(nblk)]
                state = {}

                def c0_of(i):
                    return max(0, 128 * i - 512 * j)

                def start_chunk():
                    state["yps"] = byp.tile([128, 512], F32, tag="yps",
                                            name="yps")
                    state["esum"] = epool.tile([128, 512], F16, bufs=3, tag="esum",
                                               name="esum")

                def group(blocks):
                    stile = bst.tile([128, 512], F32, tag="st", bufs=3)
                    e = epool.tile([128, 512], F16, tag="e", bufs=4)
                    for b, i in enumerate(blocks):
                        c0 = c0_of(i)
                        mm = nc.tensor.matmul(
                            stile[:, b * 512 + c0:(b + 1) * 512],
                            kTh[:, i * 128:(i + 1) * 128],
                            qTh[:, j * 512 + c0:(j + 1) * 512],
                            start=True, stop=True)
                        if qfence and not state.get("qfenced"):
                            state["qfenced"] = True
                            d = qfence.get(j)
                            if d:
                                mm.ins.add_dependency(
                                    d,
                                    bass_rust.DependencyInfo.SYNC_ONLY)
                        if i >= 4 * j:
                            nc.vector.tensor_add(
                                stile[:, b * 512 + c0:b * 512 + c0 + 128],
                                stile[:, b * 512 + c0:b * 512 + c0 + 128],
                                trimask[:])
                    # exp over the full group width: cols [b*512, b*512+c0)
                    # of diagonal blocks are exp of stale PSUM, but every
                    # downstream read (esum add, y matmul) narrows past c0.
                    w = 512 * len(blocks)
                    nc.scalar.activation(
                        e[:, 0:w], stile[:, 0:w],
                        mybir.ActivationFunctionType.Exp,
                        bias=ebias[:], scale=SCALE)
                    esum = state["esum"]
                    for b, i in enumerate(blocks):
                        c0 = c0_of(i)
                        if i == 0:
                            nc.vector.tensor_copy(
                                esum[:], e[:, 0:512])
                        else:
                            nc.vector.tensor_add(
                                esum[:, c0:], esum[:, c0:],
                                e[:, b * 512 + c0:(b + 1) * 512])
                    state["e"] = e
                    state["eblocks"] = blocks

                def ymm():
                    e = state["e"]
                    for b, i in enumerate(state["eblocks"]):
                        c0 = c0_of(i)
                        mm = nc.tensor.matmul(
                            state["yps"][:, c0:],
                            v_sb[:, i, h * 128:(h + 1) * 128],
                            e[:, b * 512 + c0:(b + 1) * 512],
                            start=(i == 0), stop=(i == nblk - 1),
                            skip_group_check=True)
                        if i == 0 and shared["last_drain"]:
                            # WAR: byp is single-buffered and tile does
                            # not track the drain's PSUM read
                            mm.ins.add_dependency(
                                shared["last_drain"],
                                bass_rust.DependencyInfo.SYNC_ONLY)

                def drain_chunk():
                    sums = bsm.tile([128, 512], F32, tag="sums",
                                    name="sums")
                    nc.tensor.matmul(sums[:], onesv[:], state["esum"][:],
                                     start=True, stop=True)
                    # sums >= the diagonal term 2^-12*exp(|q|^2*scale),
                    # which is far above fp16/f32 underflow: no clamp.
                    recip = drain.tile([128, 512], F32, tag="recip",
                                       bufs=1)
                    nc.vector.reciprocal(recip[:], sums[:])
                    mul = nc.vector.tensor_mul(
                        yT_sb[:, h, j * 512:(j + 1) * 512],
                        state["yps"][:], recip[:])
                    shared["last_drain"] = mul.ins.name
                    shared["drains"][(h, j)] = mul.ins.name

                for g in range(len(groups)):
                    def q_g(g=g):
                        if g == 0:
                            start_chunk()
                        else:
                            ymm()
                        group(groups[g])
                    quanta.append(q_g)

                def q_last():
                    ymm()
                    drain_chunk()
                quanta.append(q_last)
                return quanta

            return [make_chunk(j) for j in range(TCH)]

        # ------------------------- phase A2: v -------------------------
        def emit_a2(xT_sb, wvres, vps):
            wvt = wvres.tile([128, CCH, F], F16)
            for cc4 in range(4):
                nc.gpsimd.dma_start(
                    out=wvt[:, cc4 * 4:(cc4 + 1) * 4, 0:512],
                    in_=wvr[:, cc4 * 4:(cc4 + 1) * 4, 0:512])
            nc.scalar.dma_start(out=wvt[:, :, 512:1024],
                                in_=wvr[:, :, 512:1024])
            for j in range(2):
                for tt in range(TT):
                    ps = vps.tile([128, 512], F32, tag="vps")
                    for cc in range(CCH):
                        nc.tensor.matmul(
                            ps[:],
                            xT_sb[:, cc, tt * 128:(tt + 1) * 128],
                            wvt[:, cc, j * 512:(j + 1) * 512],
                            start=(cc == 0), stop=(cc == CCH - 1))
                    nc.scalar.copy(
                        v_sb[:, tt, j * 512:(j + 1) * 512], ps[:])

        # ----------------- phase A1 quanta for one head ----------------
        whold = {}

        def load_w(wstream, dram, h, nm):
            wt = wstream.tile([128, CCH, 128], F16, tag="w", name=nm)
            nc.sync.dma_start(out=wt[:], in_=dram[:, h])
            return wt

        def a1_quanta(wstream, xT_sb, aps, aev, h, chunked_q=False):
            wqh, wkh = whold.pop(h)
            qTh = qk.tile([128, T], F16, tag="qT")
            kTh = qk.tile([128, T], F16, tag="kT")
            head_tiles[h] = (qTh, kTh)

            def make_tensor(idx, wt, dst, dram, chunk_rope=False):
                state = {}
                quanta = []

                def start_tensor():
                    state["rawT"] = aev.tile([128, T], F16, tag="raw",
                                             name="rawT", bufs=2)
                    if chunk_rope:
                        state["rotT"] = aev.tile([128, T], F16, tag="rot",
                                                 name="rotT", bufs=2)

                for tc_i in range(TCH):
                    def unit(tc_i=tc_i):
                        if tc_i == 0:
                            start_tensor()
                        ts = slice(tc_i * 512, (tc_i + 1) * 512)
                        ps = aps.tile([128, 512], F32, tag="aps")
                        for cc in range(CCH):
                            nc.tensor.matmul(
                                ps[:], wt[:, cc, :], xT_sb[:, cc, ts],
                                start=(cc == 0), stop=(cc == CCH - 1))
                        nc.scalar.copy(state["rawT"][:, ts], ps[:])
                        if chunk_rope:
                            # issue the half-swap DMAs here (on two
                            # queues) so they hide under the previous
                            # attention chunk instead of blocking the
                            # rope muls later
                            rawT, rotT = state["rawT"], state["rotT"]
                            nc.gpsimd.dma_start(out=rotT[0:64, ts],
                                                in_=rawT[64:128, ts])
                            nc.scalar.dma_start(out=rotT[64:128, ts],
                                                in_=rawT[0:64, ts])
                    quanta.append(unit)
                    if chunk_rope:
                        def rope_chunk(tc_i=tc_i):
                            ts = slice(tc_i * 512, (tc_i + 1) * 512)
                            rawT, rotT = state["rawT"], state["rotT"]
                            nc.vector.tensor_mul(
                                dst[:, ts], rawT[:, ts], cosT_sb[:, ts])
                            nc.vector.tensor_mul(
                                rotT[:, ts], rotT[:, ts], sinS_sb[:, ts])
                            add = nc.vector.tensor_add(
                                dst[:, ts], dst[:, ts], rotT[:, ts])
                            shared["qrope"][tc_i] = add.ins.name
                        quanta.append(rope_chunk)

                if not chunk_rope:
                    def finish():
                        # prefetch this tensor's weights for head h+1
                        # (its predecessor buffer is free now)
                        if h + 1 < HPC:
                            whold.setdefault(h + 1, [None, None])[idx] = \
                                load_w(wstream, dram, h + 1,
                                       f"w{idx}_{h + 1}")
                        rawT = state["rawT"]
                        rotT = aev.tile([128, T], F16, tag="rot", bufs=2)
                        nc.gpsimd.dma_start(out=rotT[0:64, :],
                                            in_=rawT[64:128, :])
                        nc.gpsimd.dma_start(out=rotT[64:128, :],
                                            in_=rawT[0:64, :])
                        nc.vector.tensor_mul(dst[:], rawT[:], cosT_sb[:])
                        nc.vector.tensor_mul(rotT[:], rotT[:], sinS_sb[:])
                        nc.vector.tensor_add(dst[:], dst[:], rotT[:])
                    quanta.append(finish)
                return quanta

            if chunked_q:
                # k first (attention chunk j needs all k blocks <= j),
                # then q chunk-by-chunk with per-chunk RoPE so chunk j's
                # attention can interleave right after q chunk j
                return (make_tensor(1, wkh, kTh, wkr)
                        + make_tensor(0, wqh, qTh, wqr, chunk_rope=True))
            return (make_tensor(0, wqh, qTh, wqr)
                    + make_tensor(1, wkh, kTh, wkr))

        def merge_emit(a_list, b_list, frac=1.0):
            """Emit two quantum streams proportionally interleaved.
            With frac < 1 the b stream finishes when the a stream is at
            that fraction, leaving an a-only tail that hides the b
            drain latency under projection matmuls."""
            na, nb = len(a_list), len(b_list)
            ia = ib = 0
            while ia < na or ib < nb:
                if (ib < nb and ib * frac * max(na, 1) <=
                        ia * max(nb, 1)) or ia >= na:
                    b_list[ib]()
                    ib += 1
                else:
                    a_list[ia]()
                    ia += 1

        # ------------------------- main schedule -----------------------
        with tc.tile_pool(name="bst", bufs=1, space="PSUM") as bst, \
             tc.tile_pool(name="byp", bufs=2, space="PSUM") as byp, \
             tc.tile_pool(name="bsm", bufs=1, space="PSUM") as bsm:
            with tc.tile_pool(name="xres", bufs=1) as xres:
                xT_sb = xres.tile([128, CCH, T], F16)
                xsplits = [(0, 128), (128, 512), (512, 1024),
                           (1024, 1536), (1536, 2048)]
                for q, (lo, hi) in enumerate(xsplits):
                    eng = nc.sync if q % 2 == 0 else nc.scalar
                    eng.dma_start(
                        out=xT_sb[:, :, lo:hi],
                        in_=xTr[:, :, lo:hi])

                if "a" in phases:
                  with tc.tile_pool(name="wstream", bufs=2) as wstream:
                    whold[0] = [load_w(wstream, wqr, 0, "wq0"),
                                load_w(wstream, wkr, 0, "wk0")]
                    with tc.tile_pool(name="wvres", bufs=1) as wvres, \
                         tc.tile_pool(name="vps", bufs=2,
                                      space="PSUM") as vps:
                        emit_a2(xT_sb, wvres, vps)

                    with tc.tile_pool(name="aps", bufs=2,
                                      space="PSUM") as aps, \
                         tc.tile_pool(name="aev", bufs=2) as aev:
                        full = ("b" in phases and "c" in phases)
                        for h in range(HPC):
                            b_list = (sum(b_quanta(h - 1, bst, byp, bsm), [])
                                      if ("b" in phases and h > 0) else [])
                            if h == HPC - 1 and full:
                                # k, then q chunkwise; head-7 attention
                                # chunks interleave into its own q
                                # projection so the c phase needs no
                                # attention tail
                                a_list = a1_quanta(wstream, xT_sb, aps,
                                                   aev, h, chunked_q=True)
                                b7 = b_quanta(h, bst, byp, bsm,
                                              qfence=shared["qrope"])
                                comb = a_list[:7]
                                for j in range(TCH - 1):
                                    comb = comb + b7[j] +                                         a_list[7 + 2 * j:9 + 2 * j]
                                comb = comb + b7[TCH - 1]
                                merge_emit(comb, b_list)
                                shared["b7_done"] = True
                            else:
                                a_list = a1_quanta(wstream, xT_sb, aps,
                                                   aev, h)
                                merge_emit(a_list, b_list)

            # tail: last head's attention + output projection
            with tc.tile_pool(name="cres", bufs=1) as cres, \
                 tc.tile_pool(name="cps", bufs=2, space="PSUM") as cps, \
                 tc.tile_pool(name="cev", bufs=3) as cev:
                wo_sb = None
                if "c" in phases:
                    wo_sb = cres.tile([128, HPC, C], F16)
                    qeng = [nc.sync, nc.scalar, nc.gpsimd, nc.sync]
                    for r in range(4):
                        qeng[r].dma_start(
                            out=wo_sb[:, :, r * 512:(r + 1) * 512],
                            in_=wor[:, :, r * 512:(r + 1) * 512])

                def c_units(tc_i):
                    units = []
                    ts = slice(tc_i * 512, (tc_i + 1) * 512)
                    for ct in range(C // 128):
                        def unit(ct=ct, ts=ts, tc_i=tc_i):
                            ps = cps.tile([128, 512], F32, tag="cps")
                            for fc in range(HPC):
                                mm = nc.tensor.matmul(
                                    ps[:],
                                    wo_sb[:, fc, ct * 128:(ct + 1) * 128],
                                    yT_sb[:, fc, ts],
                                    start=(fc == 0), stop=(fc == HPC - 1))
                                if fc == 0 and ct == 0:
                                    # fence: tile's subtile RAW tracking
                                    # misses some yT reads vs the head-7
                                    # drain writes; PE program order then
                                    # fences every later c matmul
                                    d = shared["drains"].get(
                                        (HPC - 1, tc_i))
                                    if d:
                                        mm.ins.add_dependency(
                                            d,
                                            bass_rust.DependencyInfo
                                            .SYNC_ONLY)
                            ev = cev.tile([128, 512], F16, tag="cev")
                            nc.scalar.copy(ev[:], ps[:])
                            nc.sync.dma_start(
                                out=outT[ct * 128:(ct + 1) * 128, ts],
                                in_=ev[:])
                        units.append(unit)
                    return units

                if "b" in phases and "a" in phases and "c" in phases:
                    assert shared.get("b7_done")
                    for j in range(TCH):
                        for u in c_units(j):
                            u()
                elif "b" in phases and "a" in phases:
                    for q in sum(b_quanta(HPC - 1, bst, byp, bsm), []):
                        q()
                elif "c" in phases:
                    for tc_i in range(TCH):
                        for u in c_units(tc_i):
                            u()


def _elide_redundant_ldweights(nc):
    """Drop InstLdweights that reload the stationary operand already
    sitting in the PE array (identical weights AP, no intervening PE
    instruction that clobbers the array).  The cost-model sim treats
    weight loads as free, but hardware pays ~50ns per 128-column load;
    pairing matmuls on the same stationary and eliding the second load
    recovers that time.  Deps of a dropped load move onto the next PE
    instruction; deps of other instructions pointing at a dropped load
    are remapped the same way."""
    n_elided = 0
    for fn in nc.m.functions:
        remap = {}
        for blk in fn.blocks:
            cur_sig = None
            keep = []
            pending = []          # (dep_name, DependencyInfo) from drops
            unresolved = []       # dropped names awaiting a successor
            for inst in blk.instructions:
                t = type(inst).__name__
                if t == "InstLdweights":
                    sig = (str(inst.ins[0]),
                           str(getattr(inst, "perf_mode", None)),
                           bool(getattr(inst, "is_transpose", False) or False),
                           str(getattr(inst, "tile_position", None)))
                    if sig == cur_sig:
                        for d in inst.sync_dependency_names():
                            pending.append((d, inst.get_dependency_info(d)))
                        for d in inst.nosync_dependency_names():
                            pending.append((d, inst.get_dependency_info(d)))
                        unresolved.append(inst.name)
                        n_elided += 1
                        continue
                    cur_sig = sig
                elif t == "InstMatmult":
                    if getattr(inst, "is_transpose", False) or inst.ldweights:
                        cur_sig = None
                if getattr(inst, "engine", None) == mybir.EngineType.PE and \
                        t in ("InstMatmult", "InstLdweights"):
                    if pending:
                        have = set(inst.sync_dependency_names()) | \
                            set(inst.nosync_dependency_names())
                        for d, info in pending:
                            if d not in have and d != inst.name:
                                inst.add_dependency(d, info)
                                have.add(d)
                        pending = []
                    for name in unresolved:
                        remap[name] = inst.name
                    unresolved = []
                keep.append(inst)
            assert not pending and not unresolved, \
                "dropped Ldweights with no PE successor in block"
            blk.instructions = keep
        if remap:
            for blk in fn.blocks:
                for inst in blk.instructions:
                    deps = set(inst.sync_dependency_names()) | \
                        set(inst.nosync_dependency_names())
                    hits = deps & remap.keys()
                    if not hits:
                        continue
                    m = {}
                    for d in hits:
                        tgt = remap[d]
                        if tgt == inst.name or tgt in deps:
                            inst.try_remove_dependency(d)
                        else:
                            m[d] = tgt
                    if m:
                        inst.remap_dependency_names(m)
    return n_elided


_CACHE = {}


def _get_program():
    if "nc" not in _CACHE:
        _CACHE["nc"] = _build_program()
    return _CACHE["nc"]


def _make_in_maps(x, Wqkv, Wout):
    x = np.asarray(x, dtype=np.float32)
    Wqkv = np.asarray(Wqkv, dtype=np.float32)
    Wout = np.asarray(Wout, dtype=np.float32)
    in_maps = []
    for core in range(NCORES):
        b, g = core // 2, core % 2
        fs = slice(g * F, (g + 1) * F)
        wq = Wqkv[:, fs].astype(np.float16)
        wk = Wqkv[:, C:][:, fs].astype(np.float16)
        wv = Wqkv[:, 2 * C:][:, fs].astype(np.float16)
        wo = Wout[fs, :].astype(np.float16)
        in_maps.append({
            "xT": np.ascontiguousarray(
                x[b].T.astype(np.float16)
                .reshape(CCH, 128, T).transpose(1, 0, 2)),
            "wq": np.ascontiguousarray(
                wq.reshape(CCH, 128, HPC, 128).transpose(1, 2, 0, 3)),
            "wk": np.ascontiguousarray(
                wk.reshape(CCH, 128, HPC, 128).transpose(1, 2, 0, 3)),
            "wv": np.ascontiguousarray(
                wv.reshape(CCH, 128, F).transpose(1, 0, 2)),
            "wout": np.ascontiguousarray(
                wo.reshape(HPC, 128, C).transpose(1, 0, 2)),
        })
    return in_maps


def run_sharded(x, Wqkv, Wout, trace=False):
    """Run the SPMD program; returns (out [B,T,C], BassKernelResults)."""
    nc = _get_program()
    res = run_bass_kernel_spmd(
        nc, _make_in_maps(x, Wqkv, Wout), list(range(NCORES)), trace=trace)
    out = np.empty((B, T, C), dtype=np.float32)
    for b in range(B):
        acc = (res.results[2 * b]["outT"].astype(np.float32)
               + res.results[2 * b + 1]["outT"].astype(np.float32))
        out[b] = acc.T
    return out, res


def kernel(x, Wqkv, Wout):
    out, _ = run_sharded(x, Wqkv, Wout, trace=False)
    return out

